# revision 8
# baseline (speedup 1.0000x reference)
"""Trainium2 Bass kernel for CNN+GRU actor-critic (T=32, N=16 envs, H=512).

Sharding: data-parallel over envs — each of the 8 cores processes 2 envs
x 32 timesteps = 64 images through the CNN trunk + fc + input-gate GEMM,
then runs the GRU recurrence locally for its 2 envs, then the actor/critic
heads. All parameters are replicated. No collectives; the host scatters
inputs and gathers outputs.

Layout strategy (per core):
  * conv1 (8x8 s4) consumes host-prepared "phase planes"
      P2[p=(ry,c,dy,dx), Y, X] = img[c, 4(Y+ry)+dy, 4X+dx]
    so the 256-term contraction becomes 2 accumulating K=128 matmuls (rx).
    The stationary is padded to M=128 output columns (g',o) so conv1's
    output lands directly in conv2's phase layout [(dy2,dx2,o), Y2, X2].
  * conv2 (4x4 s2): 4 accumulating K=128 matmuls over (ry,rx); stationary
    columns duplicated (kyg,o) so eviction produces the row-shifted pair
    QQ[(kyg,c), Yq, X] = relu(conv2)[c, Yq+kyg, X] that conv3 needs.
  * conv3 (3x3 s1): ky in {0,1} packed on partitions (K=128), ky=2 as a
    K=64 matmul; stationary columns duplicated (g4,o) so eviction writes
    DUP[(g4,c), yxq, img] = relu(conv3)[c, 4*yxq+g4], which is exactly the
    fc moving operand for k-chunks of 128 = (4 spatial positions x 32 ch).
  * fc / gi: plain chunked GEMMs, images batched in the moving free dim.
  * GRU scan, "form S": stationary = w_hh.T chunks (48 LDW/step, bf16 FWL),
    moving = h.T [128,2]; gates land on partitions -> cheap pointwise.
  * heads: lhsT = GRU outputs [128, 64 imgs], moving = [actor;critic].T
    [128,7] fp32; log-softmax / entropy / gather pointwise on [64,7].

Matmul inputs are bf16 (PSUM accumulation fp32); scan pointwise, GI and
heads are fp32. Validated vs the jax reference at rel err ~1e-3 (value),
~5e-3 (states_out, which has ~1e-3 scale).
"""
import numpy as np
import ml_dtypes

import concourse.bass as bass
import concourse.tile as tile
from concourse import bacc, mybir
from concourse.bass_utils import run_bass_kernel_spmd

T, N, C, HW, A, H = 32, 16, 4, 124, 6, 512
NCORES = 8
EPC = N // NCORES            # 2 envs per core
IPC = T * EPC                # 64 images per core
BF16 = ml_dtypes.bfloat16

F32 = mybir.dt.float32
BF = mybir.dt.bfloat16
AF = mybir.ActivationFunctionType
OP = mybir.AluOpType
AX = mybir.AxisListType

_cache = {}


# ----------------------------------------------------------------------------
# device program
# ----------------------------------------------------------------------------

def build_program(reps: int = 1):
    nc = bacc.Bacc("TRN2", target_bir_lowering=False, debug=False)

    di = {}

    def inp(name, shape, dt):
        di[name] = nc.dram_tensor(name, list(shape), dt, kind="ExternalInput")
        return di[name]

    def outp(name, shape, dt):
        di[name] = nc.dram_tensor(name, list(shape), dt, kind="ExternalOutput")
        return di[name]

    d_p2 = inp("p2", [IPC, 128, 930], BF)          # (30*31 free)
    d_s1 = inp("s1", [8, 128, 128], BF)            # (g*2+rx)
    d_w2 = inp("w2", [4, 128, 128], BF)            # (ry*2+rx)
    d_w3a = inp("w3a", [3, 128, 128], BF)          # kx
    d_w3b = inp("w3b", [3, 64, 128], BF)           # kx
    d_fcw = inp("fcw", [144, 128, 128], BF)        # (j*4+mc)
    d_giw = inp("giw", [48, 128, 128], BF)         # (hc*12+gc)
    d_whh = inp("whh", [48, 128, 128], BF)         # (hc*12+gc)
    d_acw = inp("acw", [4, 128, 7], F32)           # hc
    d_b1 = inp("b1", [128, 1], F32)
    d_b2 = inp("b2", [128, 1], F32)
    d_b3 = inp("b3", [128, 1], F32)
    d_fcb = inp("fcb", [128, 4], F32)
    d_gib = inp("gib", [128, 12], F32)
    d_bhhn = inp("bhhn", [128, 4 * EPC], F32)      # [(gc',e)] rep, gc'=0..3
    d_mrep = inp("mrep", [128, T * 4 * EPC], F32)  # [(t,hc,e)] rep over p,hc
    d_h0 = inp("h0", [128, 4 * EPC], F32)          # [(hc,e)]
    d_am = inp("am", [IPC, A], F32)                # one-hot(action)
    d_acb = inp("acb", [IPC, 7], F32)              # [actor_b, critic_b] rep

    d_v = outp("v", [IPC, 1], F32)
    d_alp = outp("alp", [IPC, 1], F32)
    d_ent = outp("ent", [IPC, 1], F32)
    d_st = outp("st", [128, 4 * EPC], F32)

    with tile.TileContext(nc) as tc:
        from contextlib import ExitStack
        with ExitStack() as ctx:
            cpool = ctx.enter_context(tc.tile_pool(name="consts", bufs=1))
            work = ctx.enter_context(tc.tile_pool(name="work", bufs=3))
            small = ctx.enter_context(tc.tile_pool(name="small", bufs=2))
            persist = ctx.enter_context(tc.tile_pool(name="persist", bufs=1))
            pscv = ctx.enter_context(
                tc.tile_pool(name="pscv", bufs=3, space="PSUM"))
            pssm = ctx.enter_context(
                tc.tile_pool(name="pssm", bufs=4, space="PSUM"))
            pshd = ctx.enter_context(
                tc.tile_pool(name="pshd", bufs=1, space="PSUM"))

            # ---- load constants ----
            def cload(dram, shape, dt):
                t = cpool.tile(list(shape), dt, tag=dram.name)
                if len(dram.shape) == 3:
                    a, p, m = dram.shape
                    nc.sync.dma_start(
                        t[:].rearrange("p (a m) -> p a m", a=a),
                        dram.ap().rearrange("a p m -> p a m"))
                else:
                    nc.sync.dma_start(t[:], dram[:])
                return t

            s1 = cload(d_s1, [128, 8 * 128], BF)
            w2 = cload(d_w2, [128, 4 * 128], BF)
            w3a = cload(d_w3a, [128, 3 * 128], BF)
            w3b = cload(d_w3b, [64, 3 * 128], BF)
            fcw = cload(d_fcw, [128, 144 * 128], BF)
            giw = cload(d_giw, [128, 48 * 128], BF)
            whh = cload(d_whh, [128, 48 * 128], BF)
            acw = cload(d_acw, [128, 4 * 7], F32)
            b1 = cload(d_b1, [128, 1], F32)
            b2 = cload(d_b2, [128, 1], F32)
            b3 = cload(d_b3, [128, 1], F32)
            fcb = cload(d_fcb, [128, 4], F32)
            gib = cload(d_gib, [128, 12], F32)
            bhhn = cload(d_bhhn, [128, 4 * EPC], F32)
            mrep = cload(d_mrep, [128, T * 4 * EPC], F32)
            h0 = cload(d_h0, [128, 4 * EPC], F32)
            am = cload(d_am, [IPC, A], F32)
            acb = cload(d_acb, [IPC, 7], F32)

            def body(_iv=None):
                E = EPC
                DUP = persist.tile([128, 36 * IPC], BF, tag="dup")
                FC = persist.tile([128, 4 * IPC], BF, tag="fc")
                GI = persist.tile([128, 12 * IPC], F32, tag="gi")
                OUTS = persist.tile([128, 4 * T * E], F32, tag="outs")
                OUTSv = OUTS[:].rearrange("p (hc t e) -> p hc t e", hc=4, t=T)

                # ---------------- CNN trunk, per image ----------------
                for i in range(IPC):
                    p2 = work.tile([128, 930], BF, tag="p2")
                    nc.sync.dma_start(p2[:], d_p2[i])
                    p2v = p2[:].rearrange("p (y x) -> p y x", y=30)

                    ps2 = pscv.tile([128, 225], F32, tag="cv")
                    k = 0
                    for g in range(4):
                        dy2, dx2 = g // 2, g % 2
                        for rx in range(2):
                            rhs = p2v[:, dy2:30:2, dx2 + rx:31:2][:, :15, :15]
                            nc.tensor.matmul(
                                ps2[:], s1[:, bass.ts(g * 2 + rx, 128)], rhs,
                                start=(k == 0), stop=(k == 7))
                            k += 1
                    q = work.tile([128, 225], BF, tag="q")
                    nc.scalar.activation(q[:], ps2[:], AF.Relu, bias=b1[:])
                    qv = q[:].rearrange("p (y x) -> p y x", y=15)

                    ps3 = pscv.tile([128, 196], F32, tag="cv")
                    k = 0
                    for ry in range(2):
                        for rx in range(2):
                            rhs = qv[:, ry:ry + 14, rx:rx + 14]
                            nc.tensor.matmul(
                                ps3[:], w2[:, bass.ts(ry * 2 + rx, 128)], rhs,
                                start=(k == 0), stop=(k == 3))
                            k += 1
                    ps3v = ps3[:].rearrange("p (y x) -> p y x", y=14)
                    qq = work.tile([128, 196], BF, tag="qq")
                    qqv = qq[:].rearrange("p (y x) -> p y x", y=14)
                    nc.scalar.activation(qqv[0:64], ps3v[0:64], AF.Relu,
                                         bias=b2[0:64])
                    nc.scalar.activation(qqv[64:128, 0:13], ps3v[64:128, 1:14],
                                         AF.Relu, bias=b2[64:128])

                    ps4 = pscv.tile([128, 144], F32, tag="cv")
                    for kx in range(3):
                        nc.tensor.matmul(
                            ps4[:], w3a[:, bass.ts(kx, 128)],
                            qqv[:, 0:12, kx:kx + 12],
                            start=(kx == 0), stop=False)
                    for kx in range(3):
                        nc.tensor.matmul(
                            ps4[:], w3b[:, bass.ts(kx, 128)],
                            qqv[0:64, 2:14, kx:kx + 12],
                            start=False, stop=(kx == 2))
                    dupv = DUP[:].rearrange("p (j i) -> p j i", j=36)
                    ps4v = ps4[:].rearrange("p (yx) -> p yx")
                    for g4 in range(4):
                        sl = slice(g4 * 32, (g4 + 1) * 32)
                        nc.scalar.activation(
                            dupv[sl, :, i],
                            ps4[sl, g4:144:4],
                            AF.Relu, bias=b3[sl])

                # ---------------- fc ----------------
                dupv = DUP[:].rearrange("p (j i) -> p j i", j=36)
                for mc in range(4):
                    psf = pssm.tile([128, IPC], F32, tag="sm")
                    for j in range(36):
                        nc.tensor.matmul(
                            psf[:], fcw[:, bass.ts(j * 4 + mc, 128)],
                            dupv[:, j, :],
                            start=(j == 0), stop=(j == 35))
                    nc.scalar.activation(FC[:, bass.ts(mc, IPC)], psf[:],
                                         AF.Relu, bias=fcb[:, mc:mc + 1])

                # ---------------- gi ----------------
                for gc in range(12):
                    psg = pssm.tile([128, IPC], F32, tag="sm")
                    for hc in range(4):
                        nc.tensor.matmul(
                            psg[:], giw[:, bass.ts(hc * 12 + gc, 128)],
                            FC[:, bass.ts(hc, IPC)],
                            start=(hc == 0), stop=(hc == 3))
                    nc.scalar.activation(GI[:, bass.ts(gc, IPC)], psg[:],
                                         AF.Identity, bias=gib[:, gc:gc + 1])
                GIv = GI[:].rearrange("p (gc t e) -> p gc t e", gc=12, t=T)

                # ---------------- GRU scan ----------------
                mrv = mrep[:].rearrange("p (t he) -> p t he", t=T)
                for t in range(T):
                    hprev = h0[:] if t == 0 else OUTSv[:, :, t - 1, :]
                    hm = small.tile([128, 4 * E], F32, tag="hm")
                    hmv = hm[:].rearrange("p (hc e) -> p hc e", hc=4)
                    nc.vector.tensor_tensor(hm[:], hprev, mrv[:, t, :],
                                            op=OP.mult)
                    hmb = small.tile([128, 4 * E], BF, tag="hmb")
                    nc.vector.tensor_copy(hmb[:], hm[:])
                    hmbv = hmb[:].rearrange("p (hc e) -> p hc e", hc=4)

                    pss = pssm.tile([128, 12 * E], F32, tag="sm")
                    for gc in range(12):
                        for hc in range(4):
                            nc.tensor.matmul(
                                pss[:, bass.ts(gc, E)],
                                whh[:, bass.ts(hc * 12 + gc, 128)],
                                hmbv[:, hc, :],
                                start=(hc == 0), stop=(hc == 3))

                    arz = small.tile([128, 8 * E], F32, tag="arz")
                    nc.vector.tensor_tensor(
                        arz[:], pss[:, 0:8 * E],
                        GIv[:, 0:8, t, :], op=OP.add)
                    rz = small.tile([128, 8 * E], F32, tag="rz")
                    nc.scalar.activation(rz[:], arz[:], AF.Sigmoid)
                    bn = small.tile([128, 4 * E], F32, tag="bn")
                    nc.vector.tensor_tensor(
                        bn[:], pss[:, 8 * E:12 * E], bhhn[:], op=OP.add)
                    t1 = small.tile([128, 4 * E], F32, tag="t1")
                    nc.vector.tensor_tensor(t1[:], rz[:, 0:4 * E], bn[:],
                                            op=OP.mult)
                    t2 = small.tile([128, 4 * E], F32, tag="t2")
                    nc.vector.tensor_tensor(t2[:], t1[:], GIv[:, 8:12, t, :],
                                            op=OP.add)
                    nn = small.tile([128, 4 * E], F32, tag="nn")
                    nc.scalar.activation(nn[:], t2[:], AF.Tanh)
                    dd = small.tile([128, 4 * E], F32, tag="dd")
                    nc.vector.tensor_tensor(dd[:], hm[:], nn[:], op=OP.subtract)
                    e1 = small.tile([128, 4 * E], F32, tag="e1")
                    nc.vector.tensor_tensor(e1[:], rz[:, 4 * E:8 * E], dd[:],
                                            op=OP.mult)
                    nc.vector.tensor_tensor(OUTSv[:, :, t, :], nn[:], e1[:],
                                            op=OP.add)

                # ---------------- heads ----------------
                psl = pshd.tile([IPC, 7], F32, tag="hd")
                for hc in range(4):
                    nc.tensor.matmul(
                        psl[:], OUTSv[:, hc, :, :], acw[:, bass.ts(hc, 7)],
                        start=(hc == 0), stop=(hc == 3))
                lb = small.tile([IPC, 7], F32, tag="lb")
                nc.vector.tensor_tensor(lb[:], psl[:], acb[:], op=OP.add)
                mx = small.tile([IPC, 1], F32, tag="mx")
                nc.vector.tensor_reduce(mx[:], lb[:, 0:6], axis=AX.X,
                                        op=OP.max)
                sh = small.tile([IPC, 6], F32, tag="sh")
                nc.vector.tensor_scalar(sh[:], lb[:, 0:6], mx[:], None,
                                        op0=OP.subtract)
                ee = small.tile([IPC, 6], F32, tag="ee")
                ss = small.tile([IPC, 1], F32, tag="ss")
                nc.scalar.activation(ee[:], sh[:], AF.Exp, accum_out=ss[:])
                ls = small.tile([IPC, 1], F32, tag="ls")
                nc.scalar.activation(ls[:], ss[:], AF.Ln)
                logp = small.tile([IPC, 6], F32, tag="logp")
                nc.vector.tensor_scalar(logp[:], sh[:], ls[:], None,
                                        op0=OP.subtract)
                ta = small.tile([IPC, 6], F32, tag="ta")
                nc.vector.tensor_tensor(ta[:], logp[:], am[:], op=OP.mult)
                alp_t = small.tile([IPC, 1], F32, tag="alp")
                nc.vector.tensor_reduce(alp_t[:], ta[:], axis=AX.X, op=OP.add)
                rs = small.tile([IPC, 1], F32, tag="rs")
                nc.vector.reciprocal(rs[:], ss[:])
                pp = small.tile([IPC, 6], F32, tag="pp")
                nc.vector.tensor_scalar(pp[:], ee[:], rs[:], None,
                                        op0=OP.mult)
                tb = small.tile([IPC, 6], F32, tag="tb")
                nc.vector.tensor_tensor(tb[:], pp[:], logp[:], op=OP.mult)
                tbs = small.tile([IPC, 1], F32, tag="tbs")
                nc.vector.tensor_reduce(tbs[:], tb[:], axis=AX.X, op=OP.add)
                ent_t = small.tile([IPC, 1], F32, tag="ent")
                nc.vector.tensor_scalar(ent_t[:], tbs[:], -1.0, None,
                                        op0=OP.mult)
                vv = small.tile([IPC, 1], F32, tag="vv")
                nc.vector.tensor_copy(vv[:], lb[:, 6:7])

                nc.sync.dma_start(d_v[:], vv[:])
                nc.sync.dma_start(d_alp[:], alp_t[:])
                nc.sync.dma_start(d_ent[:], ent_t[:])
                nc.sync.dma_start(
                    d_st.ap().rearrange("p (hc e) -> p hc e", hc=4),
                    OUTSv[:, :, T - 1, :])

            if reps == 1:
                body()
            else:
                with tc.For_i(0, reps, 1) as _i:
                    body(_i)

    nc.compile()
    return nc


# ----------------------------------------------------------------------------
# host-side data prep (pure permutations / casts — no FLOPs beyond /255 fold)
# ----------------------------------------------------------------------------

def host_prep(inputs, states, masks, action, conv1_w, conv1_b, conv2_w,
              conv2_b, conv3_w, conv3_b, fc_w, fc_b, w_ih, w_hh, b_ih, b_hh,
              actor_w, actor_b, critic_w, critic_b):
    f32 = np.float32
    inputs = np.asarray(inputs, f32)
    states = np.asarray(states, f32)
    masks = np.asarray(masks, f32)
    action = np.asarray(action)
    w1 = np.asarray(conv1_w, f32); b1 = np.asarray(conv1_b, f32)
    w2 = np.asarray(conv2_w, f32); b2 = np.asarray(conv2_b, f32)
    w3 = np.asarray(conv3_w, f32); b3 = np.asarray(conv3_b, f32)
    fc_w = np.asarray(fc_w, f32); fc_b = np.asarray(fc_b, f32)
    w_ih = np.asarray(w_ih, f32); w_hh = np.asarray(w_hh, f32)
    b_ih = np.asarray(b_ih, f32); b_hh = np.asarray(b_hh, f32)
    actor_w = np.asarray(actor_w, f32); actor_b = np.asarray(actor_b, f32)
    critic_w = np.asarray(critic_w, f32)
    critic_b = np.asarray(critic_b, f32)

    shared = {}
    # conv1 stationaries: S1[(g,rx)][p=(ry,c,dy,dx), (g',o)]
    w1s = w1 / 255.0
    S1 = np.zeros((4, 2, 128, 128), f32)
    for g in range(4):
        for rx in range(2):
            for ry in range(2):
                for c in range(C):
                    for dy in range(4):
                        for dx in range(4):
                            p = ry * 64 + c * 16 + dy * 4 + dx
                            S1[g, rx, p, g * 32:(g + 1) * 32] = \
                                w1s[:, c, 4 * ry + dy, 4 * rx + dx]
    shared["s1"] = S1.reshape(8, 128, 128)

    W2 = np.zeros((2, 2, 128, 128), f32)
    for ry in range(2):
        for rx in range(2):
            for g in range(4):
                dy2, dx2 = g // 2, g % 2
                for c in range(32):
                    col = w2[:, c, 2 * ry + dy2, 2 * rx + dx2]
                    W2[ry, rx, g * 32 + c, 0:64] = col
                    W2[ry, rx, g * 32 + c, 64:128] = col
    shared["w2"] = W2.reshape(4, 128, 128)

    W3A = np.zeros((3, 128, 128), f32)
    W3B = np.zeros((3, 64, 128), f32)
    for kx in range(3):
        for kyg in range(2):
            for c in range(64):
                for g4 in range(4):
                    W3A[kx, kyg * 64 + c, g4 * 32:(g4 + 1) * 32] = \
                        w3[:, c, kyg, kx]
        for c in range(64):
            for g4 in range(4):
                W3B[kx, c, g4 * 32:(g4 + 1) * 32] = w3[:, c, 2, kx]
    shared["w3a"] = W3A
    shared["w3b"] = W3B

    FCW = np.zeros((36, 4, 128, 128), f32)
    for j in range(36):
        for g in range(4):
            for c in range(32):
                FCW[j, :, g * 32 + c, :] = \
                    fc_w[:, c * 144 + 4 * j + g].reshape(4, 128)
    shared["fcw"] = FCW.reshape(144, 128, 128)

    GIW = np.zeros((4, 12, 128, 128), f32)
    WHH = np.zeros((4, 12, 128, 128), f32)
    for hc in range(4):
        for gc in range(12):
            GIW[hc, gc] = w_ih[gc * 128:(gc + 1) * 128,
                               hc * 128:(hc + 1) * 128].T
            WHH[hc, gc] = w_hh[gc * 128:(gc + 1) * 128,
                               hc * 128:(hc + 1) * 128].T
    shared["giw"] = GIW.reshape(48, 128, 128)
    shared["whh"] = WHH.reshape(48, 128, 128)

    ACm = np.concatenate([actor_w, critic_w], axis=0)       # [7, 512]
    ACW = np.zeros((4, 128, 7), f32)
    for hc in range(4):
        ACW[hc] = ACm[:, hc * 128:(hc + 1) * 128].T
    shared["acw"] = ACW

    shared["b1"] = np.tile(b1, 4).reshape(128, 1)
    shared["b2"] = np.tile(b2, 2).reshape(128, 1)
    shared["b3"] = np.tile(b3, 4).reshape(128, 1)
    shared["fcb"] = fc_b.reshape(4, 128).T.copy()
    gib = b_ih + np.concatenate([b_hh[:1024], np.zeros(512, f32)])
    shared["gib"] = gib.reshape(12, 128).T.copy()
    bhhn = b_hh[1024:].reshape(4, 128).T                     # [128, gc']
    shared["bhhn"] = np.repeat(bhhn, EPC, axis=1).copy()     # [(gc',e)]
    acb = np.concatenate([actor_b, critic_b]).astype(f32)
    shared["acb"] = np.tile(acb, (IPC, 1))

    for k in ("s1", "w2", "w3a", "w3b", "fcw", "giw", "whh"):
        shared[k] = shared[k].astype(BF16)

    # per-core tensors
    x_all = inputs.reshape(T, N, C, HW, HW)
    m_all = masks.reshape(T, N)
    a_all = np.asarray(action).reshape(T, N)
    in_maps = []
    # vectorized phase-plane build for all images at once:
    # P[(t,n), p=(ry,c,dy,dx), Y, X] = x_all[t, n, c, 4Y+4ry+dy, 4X+dx]
    xb = x_all.reshape(T * N, C, 31, 4, 31, 4)     # [i, c, Yr, dy, Xr, dx]
    P_all = np.zeros((T * N, 2, C, 4, 4, 30, 31), np.float32)
    for ry in range(2):
        P_all[:, ry] = xb[:, :, ry:ry + 30].transpose(
            0, 1, 3, 5, 2, 4)                       # [i, c, dy, dx, Y, X]
    P_all = P_all.reshape(T * N, 128, 930).astype(BF16)

    for k in range(NCORES):
        envs = [EPC * k + e for e in range(EPC)]
        idx = [t * N + n for t in range(T) for n in envs]
        m = dict(shared)
        m["p2"] = np.ascontiguousarray(P_all[idx])
        h0 = np.zeros((128, 4, EPC), np.float32)
        for e in range(EPC):
            h0[:, :, e] = states[envs[e]].reshape(4, 128).T
        m["h0"] = h0.reshape(128, 4 * EPC)
        mr = np.zeros((T, 4, EPC), np.float32)
        for e in range(EPC):
            mr[:, :, e] = m_all[:, envs[e]][:, None]
        m["mrep"] = np.broadcast_to(
            mr.reshape(1, T * 4 * EPC), (128, T * 4 * EPC)).copy()
        amk = np.zeros((IPC, A), np.float32)
        for t in range(T):
            for e in range(EPC):
                amk[t * EPC + e, int(a_all[t, envs[e]])] = 1.0
        m["am"] = amk
        in_maps.append(m)
    return in_maps


def assemble(results):
    value = np.zeros((T * N, 1), np.float32)
    alp = np.zeros((T * N,), np.float32)
    ent = np.zeros((T * N,), np.float32)
    st = np.zeros((N, H), np.float32)
    for k in range(NCORES):
        r = results[k]
        v = r["v"].reshape(T, EPC)
        a = r["alp"].reshape(T, EPC)
        e = r["ent"].reshape(T, EPC)
        for t in range(T):
            for ee in range(EPC):
                row = t * N + EPC * k + ee
                value[row, 0] = v[t, ee]
                alp[row] = a[t, ee]
                ent[row] = e[t, ee]
        stk = r["st"].reshape(128, 4, EPC)       # [p, hc, e]
        for e in range(EPC):
            st[EPC * k + e] = stk[:, :, e].T.reshape(H)
    return value, alp, ent, st


def kernel(**inputs):
    if "nc" not in _cache:
        _cache["nc"] = build_program(reps=1)
    nc = _cache["nc"]
    in_maps = host_prep(**inputs)
    res = run_bass_kernel_spmd(nc, in_maps, list(range(NCORES)))
    return assemble(res.results)


# revision 9
# speedup vs baseline: 1.5708x; 1.5708x over previous
"""Trainium2 Bass kernel for CNN+GRU actor-critic (T=32, N=16 envs, H=512).

Sharding: data-parallel over envs — each of the 8 cores processes 2 envs
x 32 timesteps = 64 images through the CNN trunk + fc + input-gate GEMM,
then runs the GRU recurrence locally for its 2 envs, then the actor/critic
heads. All parameters are replicated. No collectives; the host scatters
inputs and gathers outputs.

Layout strategy (per core):
  * conv1 (8x8 s4) consumes host-prepared "phase planes"
      P2[p=(ry,c,dy,dx), Y, X] = img[c, 4(Y+ry)+dy, 4X+dx]
    so the 256-term contraction becomes 2 accumulating K=128 matmuls (rx).
    The stationary is padded to M=128 output columns (g',o) so conv1's
    output lands directly in conv2's phase layout [(dy2,dx2,o), Y2, X2].
  * conv2 (4x4 s2): 4 accumulating K=128 matmuls over (ry,rx); stationary
    columns duplicated (kyg,o) so eviction produces the row-shifted pair
    QQ[(kyg,c), Yq, X] = relu(conv2)[c, Yq+kyg, X] that conv3 needs.
  * conv3 (3x3 s1): ky in {0,1} packed on partitions (K=128), ky=2 as a
    K=64 matmul; stationary columns duplicated (g4,o) so eviction writes
    DUP[(g4,c), yxq, img] = relu(conv3)[c, 4*yxq+g4], which is exactly the
    fc moving operand for k-chunks of 128 = (4 spatial positions x 32 ch).
  * fc / gi: plain chunked GEMMs, images batched in the moving free dim.
  * GRU scan, "form S": stationary = w_hh.T chunks (48 LDW/step, bf16 FWL),
    moving = h.T [128,2]; gates land on partitions -> cheap pointwise.
  * heads: lhsT = GRU outputs [128, 64 imgs], moving = [actor;critic].T
    [128,7] fp32; log-softmax / entropy / gather pointwise on [64,7].

Matmul inputs are bf16 (PSUM accumulation fp32); scan pointwise, GI and
heads are fp32. Validated vs the jax reference at rel err ~1e-3 (value),
~5e-3 (states_out, which has ~1e-3 scale).
"""
import numpy as np
import ml_dtypes

import concourse.bass as bass
import concourse.tile as tile
from concourse import bacc, mybir
from concourse.bass_utils import run_bass_kernel_spmd

T, N, C, HW, A, H = 32, 16, 4, 124, 6, 512
NCORES = 8
EPC = N // NCORES            # 2 envs per core
IPC = T * EPC                # 64 images per core
BF16 = ml_dtypes.bfloat16

F32 = mybir.dt.float32
BF = mybir.dt.bfloat16
AF = mybir.ActivationFunctionType
OP = mybir.AluOpType
AX = mybir.AxisListType

_cache = {}


# ----------------------------------------------------------------------------
# device program
# ----------------------------------------------------------------------------

def build_program(reps: int = 1, n_imgs: int = IPC, do_fc: bool = True, do_scan: bool = True, do_heads: bool = True):
    nc = bacc.Bacc("TRN2", target_bir_lowering=False, debug=False)

    di = {}

    def inp(name, shape, dt):
        di[name] = nc.dram_tensor(name, list(shape), dt, kind="ExternalInput")
        return di[name]

    def outp(name, shape, dt):
        di[name] = nc.dram_tensor(name, list(shape), dt, kind="ExternalOutput")
        return di[name]

    d_p2 = inp("p2", [IPC, 128, 930], BF)          # (30*31 free)
    d_s1 = inp("s1", [8, 128, 128], BF)            # (g*2+rx)
    d_w2 = inp("w2", [4, 128, 128], BF)            # (ry*2+rx)
    d_w3a = inp("w3a", [3, 128, 128], BF)          # kx
    d_w3b = inp("w3b", [3, 64, 128], BF)           # kx
    d_fcw = inp("fcw", [144, 128, 128], BF)        # (j*4+mc)
    d_giw = inp("giw", [48, 128, 128], BF)         # (hc*12+gc)
    d_whh = inp("whh", [48, 128, 128], BF)         # (hc*12+gc)
    d_acw = inp("acw", [4, 128, 7], F32)           # hc
    d_b1 = inp("b1", [128, 1], F32)
    d_b2 = inp("b2", [128, 1], F32)
    d_b3 = inp("b3", [128, 1], F32)
    d_fcb = inp("fcb", [128, 4], F32)
    d_gib = inp("gib", [128, 12], F32)
    d_bhhn = inp("bhhn", [128, 4 * EPC], F32)      # [(gc',e)] rep, gc'=0..3
    d_mrep = inp("mrep", [128, T * 4 * EPC], F32)  # [(t,hc,e)] rep over p,hc
    d_h0 = inp("h0", [128, 4 * EPC], F32)          # [(hc,e)]
    d_am = inp("am", [IPC, A], F32)                # one-hot(action)
    d_acb = inp("acb", [IPC, 7], F32)              # [actor_b, critic_b] rep

    d_v = outp("v", [IPC, 1], F32)
    d_alp = outp("alp", [IPC, 1], F32)
    d_ent = outp("ent", [IPC, 1], F32)
    d_st = outp("st", [128, 4 * EPC], F32)

    with tile.TileContext(nc) as tc:
        from contextlib import ExitStack
        with ExitStack() as ctx:
            cpool = ctx.enter_context(tc.tile_pool(name="consts", bufs=1))
            work = ctx.enter_context(tc.tile_pool(name="work", bufs=3))
            small = ctx.enter_context(tc.tile_pool(name="small", bufs=2))
            persist = ctx.enter_context(tc.tile_pool(name="persist", bufs=1))
            pscv = ctx.enter_context(
                tc.tile_pool(name="pscv", bufs=3, space="PSUM"))
            pssm = ctx.enter_context(
                tc.tile_pool(name="pssm", bufs=4, space="PSUM"))
            pshd = ctx.enter_context(
                tc.tile_pool(name="pshd", bufs=1, space="PSUM"))

            # ---- load constants ----
            def cload(dram, shape, dt):
                t = cpool.tile(list(shape), dt, tag=dram.name)
                if len(dram.shape) == 3:
                    a, p, m = dram.shape
                    nc.sync.dma_start(
                        t[:].rearrange("p (a m) -> p a m", a=a),
                        dram.ap().rearrange("a p m -> p a m"))
                else:
                    nc.sync.dma_start(t[:], dram[:])
                return t

            s1 = cload(d_s1, [128, 8 * 128], BF)
            w2 = cload(d_w2, [128, 4 * 128], BF)
            w3a = cload(d_w3a, [128, 3 * 128], BF)
            w3b = cload(d_w3b, [64, 3 * 128], BF)
            fcw = cload(d_fcw, [128, 144 * 128], BF)
            giw = cload(d_giw, [128, 48 * 128], BF)
            whh = cload(d_whh, [128, 48 * 128], BF)
            acw = cload(d_acw, [128, 4 * 7], F32)
            b1 = cload(d_b1, [128, 1], F32)
            b2 = cload(d_b2, [128, 1], F32)
            b3 = cload(d_b3, [128, 1], F32)
            fcb = cload(d_fcb, [128, 4], F32)
            gib = cload(d_gib, [128, 12], F32)
            bhhn = cload(d_bhhn, [128, 4 * EPC], F32)
            mrep = cload(d_mrep, [128, T * 4 * EPC], F32)
            h0 = cload(d_h0, [128, 4 * EPC], F32)
            am = cload(d_am, [IPC, A], F32)
            acb = cload(d_acb, [IPC, 7], F32)

            def body(_iv=None):
                E = EPC
                DUP = persist.tile([128, 36 * IPC], BF, tag="dup")
                FC = persist.tile([128, 4 * IPC], BF, tag="fc")
                GI = persist.tile([128, 12 * IPC], F32, tag="gi")
                OUTS = persist.tile([128, 4 * T * E], F32, tag="outs")
                OUTSv = OUTS[:].rearrange("p (hc t e) -> p hc t e", hc=4, t=T)

                # ---------------- CNN trunk, per image ----------------
                for i in range(n_imgs):
                    p2 = work.tile([128, 930], BF, tag="p2")
                    nc.sync.dma_start(p2[:], d_p2[i])
                    p2v = p2[:].rearrange("p (y x) -> p y x", y=30)

                    ps2 = pscv.tile([128, 225], F32, tag="cv")
                    k = 0
                    for g in range(4):
                        dy2, dx2 = g // 2, g % 2
                        for rx in range(2):
                            rhs = p2v[:, dy2:30:2, dx2 + rx:31:2][:, :15, :15]
                            nc.tensor.matmul(
                                ps2[:], s1[:, bass.ts(g * 2 + rx, 128)], rhs,
                                start=(k == 0), stop=(k == 7))
                            k += 1
                    q = work.tile([128, 225], BF, tag="q")
                    nc.scalar.activation(q[:], ps2[:], AF.Relu, bias=b1[:])
                    qv = q[:].rearrange("p (y x) -> p y x", y=15)

                    ps3 = pscv.tile([128, 196], F32, tag="cv")
                    k = 0
                    for ry in range(2):
                        for rx in range(2):
                            rhs = qv[:, ry:ry + 14, rx:rx + 14]
                            nc.tensor.matmul(
                                ps3[:], w2[:, bass.ts(ry * 2 + rx, 128)], rhs,
                                start=(k == 0), stop=(k == 3))
                            k += 1
                    ps3v = ps3[:].rearrange("p (y x) -> p y x", y=14)
                    qq = work.tile([128, 196], BF, tag="qq")
                    qqv = qq[:].rearrange("p (y x) -> p y x", y=14)
                    nc.scalar.activation(qqv[0:64], ps3v[0:64], AF.Relu,
                                         bias=b2[0:64])
                    nc.scalar.activation(qqv[64:128, 0:13], ps3v[64:128, 1:14],
                                         AF.Relu, bias=b2[64:128])

                    ps4 = pscv.tile([128, 144], F32, tag="cv")
                    for kx in range(3):
                        nc.tensor.matmul(
                            ps4[:], w3a[:, bass.ts(kx, 128)],
                            qqv[:, 0:12, kx:kx + 12],
                            start=(kx == 0), stop=False)
                    for kx in range(3):
                        nc.tensor.matmul(
                            ps4[:], w3b[:, bass.ts(kx, 128)],
                            qqv[0:64, 2:14, kx:kx + 12],
                            start=False, stop=(kx == 2))
                    dupv = DUP[:].rearrange("p (j i) -> p j i", j=36)
                    ps4v = ps4[:].rearrange("p (yx) -> p yx")
                    for g4 in range(4):
                        sl = slice(g4 * 32, (g4 + 1) * 32)
                        nc.scalar.activation(
                            dupv[sl, :, i],
                            ps4[sl, g4:144:4],
                            AF.Relu, bias=b3[sl])

                # ---------------- fc ----------------
                if not do_fc:
                    nc.sync.dma_start(d_v[:], am[:, 0:1])
                    nc.sync.dma_start(d_alp[:], am[:, 0:1])
                    nc.sync.dma_start(d_ent[:], am[:, 0:1])
                    nc.sync.dma_start(d_st[:], mrep[:, 0:4 * EPC])
                    return
                dupv = DUP[:].rearrange("p (j i) -> p j i", j=36)
                for mc in range(4):
                    psf = pssm.tile([128, IPC], F32, tag="sm")
                    for j in range(36):
                        nc.tensor.matmul(
                            psf[:], fcw[:, bass.ts(j * 4 + mc, 128)],
                            dupv[:, j, :],
                            start=(j == 0), stop=(j == 35))
                    nc.scalar.activation(FC[:, bass.ts(mc, IPC)], psf[:],
                                         AF.Relu, bias=fcb[:, mc:mc + 1])

                # ---------------- gi ----------------
                for gc in range(12):
                    psg = pssm.tile([128, IPC], F32, tag="sm")
                    for hc in range(4):
                        nc.tensor.matmul(
                            psg[:], giw[:, bass.ts(hc * 12 + gc, 128)],
                            FC[:, bass.ts(hc, IPC)],
                            start=(hc == 0), stop=(hc == 3))
                    nc.scalar.activation(GI[:, bass.ts(gc, IPC)], psg[:],
                                         AF.Identity, bias=gib[:, gc:gc + 1])
                GIv = GI[:].rearrange("p (gc t e) -> p gc t e", gc=12, t=T)

                # ---------------- GRU scan ----------------
                if not do_scan:
                    nc.sync.dma_start(d_v[:], am[:, 0:1])
                    nc.sync.dma_start(d_alp[:], am[:, 0:1])
                    nc.sync.dma_start(d_ent[:], am[:, 0:1])
                    nc.sync.dma_start(d_st[:], mrep[:, 0:4 * EPC])
                    return
                mrv = mrep[:].rearrange("p (t he) -> p t he", t=T)
                for t in range(T):
                    hprev = h0[:] if t == 0 else OUTSv[:, :, t - 1, :]
                    hm = small.tile([128, 4 * E], F32, tag="hm")
                    hmv = hm[:].rearrange("p (hc e) -> p hc e", hc=4)
                    nc.vector.tensor_tensor(hm[:], hprev, mrv[:, t, :],
                                            op=OP.mult)
                    hmb = small.tile([128, 4 * E], BF, tag="hmb")
                    nc.vector.tensor_copy(hmb[:], hm[:])
                    hmbv = hmb[:].rearrange("p (hc e) -> p hc e", hc=4)

                    pss = pssm.tile([128, 12 * E], F32, tag="sm")
                    for gc in range(12):
                        for hc in range(4):
                            nc.tensor.matmul(
                                pss[:, bass.ts(gc, E)],
                                whh[:, bass.ts(hc * 12 + gc, 128)],
                                hmbv[:, hc, :],
                                start=(hc == 0), stop=(hc == 3))

                    arz = small.tile([128, 8 * E], F32, tag="arz")
                    nc.vector.tensor_tensor(
                        arz[:], pss[:, 0:8 * E],
                        GIv[:, 0:8, t, :], op=OP.add)
                    rz = small.tile([128, 8 * E], F32, tag="rz")
                    nc.scalar.activation(rz[:], arz[:], AF.Sigmoid)
                    bn = small.tile([128, 4 * E], F32, tag="bn")
                    nc.vector.tensor_tensor(
                        bn[:], pss[:, 8 * E:12 * E], bhhn[:], op=OP.add)
                    t1 = small.tile([128, 4 * E], F32, tag="t1")
                    nc.vector.tensor_tensor(t1[:], rz[:, 0:4 * E], bn[:],
                                            op=OP.mult)
                    t2 = small.tile([128, 4 * E], F32, tag="t2")
                    nc.vector.tensor_tensor(t2[:], t1[:], GIv[:, 8:12, t, :],
                                            op=OP.add)
                    nn = small.tile([128, 4 * E], F32, tag="nn")
                    nc.scalar.activation(nn[:], t2[:], AF.Tanh)
                    dd = small.tile([128, 4 * E], F32, tag="dd")
                    nc.vector.tensor_tensor(dd[:], hm[:], nn[:], op=OP.subtract)
                    e1 = small.tile([128, 4 * E], F32, tag="e1")
                    nc.vector.tensor_tensor(e1[:], rz[:, 4 * E:8 * E], dd[:],
                                            op=OP.mult)
                    nc.vector.tensor_tensor(OUTSv[:, :, t, :], nn[:], e1[:],
                                            op=OP.add)

                # ---------------- heads ----------------
                if not do_heads:
                    nc.sync.dma_start(d_v[:], am[:, 0:1])
                    nc.sync.dma_start(d_alp[:], am[:, 0:1])
                    nc.sync.dma_start(d_ent[:], am[:, 0:1])
                    nc.sync.dma_start(
                        d_st.ap().rearrange("p (hc e) -> p hc e", hc=4),
                        OUTSv[:, :, T - 1, :])
                    return
                psl = pshd.tile([IPC, 7], F32, tag="hd")
                for hc in range(4):
                    nc.tensor.matmul(
                        psl[:], OUTSv[:, hc, :, :], acw[:, bass.ts(hc, 7)],
                        start=(hc == 0), stop=(hc == 3))
                lb = small.tile([IPC, 7], F32, tag="lb")
                nc.vector.tensor_tensor(lb[:], psl[:], acb[:], op=OP.add)
                mx = small.tile([IPC, 1], F32, tag="mx")
                nc.vector.tensor_reduce(mx[:], lb[:, 0:6], axis=AX.X,
                                        op=OP.max)
                sh = small.tile([IPC, 6], F32, tag="sh")
                nc.vector.tensor_scalar(sh[:], lb[:, 0:6], mx[:], None,
                                        op0=OP.subtract)
                ee = small.tile([IPC, 6], F32, tag="ee")
                ss = small.tile([IPC, 1], F32, tag="ss")
                nc.scalar.activation(ee[:], sh[:], AF.Exp, accum_out=ss[:])
                ls = small.tile([IPC, 1], F32, tag="ls")
                nc.scalar.activation(ls[:], ss[:], AF.Ln)
                logp = small.tile([IPC, 6], F32, tag="logp")
                nc.vector.tensor_scalar(logp[:], sh[:], ls[:], None,
                                        op0=OP.subtract)
                ta = small.tile([IPC, 6], F32, tag="ta")
                nc.vector.tensor_tensor(ta[:], logp[:], am[:], op=OP.mult)
                alp_t = small.tile([IPC, 1], F32, tag="alp")
                nc.vector.tensor_reduce(alp_t[:], ta[:], axis=AX.X, op=OP.add)
                rs = small.tile([IPC, 1], F32, tag="rs")
                nc.vector.reciprocal(rs[:], ss[:])
                pp = small.tile([IPC, 6], F32, tag="pp")
                nc.vector.tensor_scalar(pp[:], ee[:], rs[:], None,
                                        op0=OP.mult)
                tb = small.tile([IPC, 6], F32, tag="tb")
                nc.vector.tensor_tensor(tb[:], pp[:], logp[:], op=OP.mult)
                tbs = small.tile([IPC, 1], F32, tag="tbs")
                nc.vector.tensor_reduce(tbs[:], tb[:], axis=AX.X, op=OP.add)
                ent_t = small.tile([IPC, 1], F32, tag="ent")
                nc.vector.tensor_scalar(ent_t[:], tbs[:], -1.0, None,
                                        op0=OP.mult)
                vv = small.tile([IPC, 1], F32, tag="vv")
                nc.vector.tensor_copy(vv[:], lb[:, 6:7])

                nc.sync.dma_start(d_v[:], vv[:])
                nc.sync.dma_start(d_alp[:], alp_t[:])
                nc.sync.dma_start(d_ent[:], ent_t[:])
                nc.sync.dma_start(
                    d_st.ap().rearrange("p (hc e) -> p hc e", hc=4),
                    OUTSv[:, :, T - 1, :])

            if reps == 1:
                body()
            else:
                with tc.For_i(0, reps, 1) as _i:
                    body(_i)

    nc.compile()
    return nc


# ----------------------------------------------------------------------------
# host-side data prep (pure permutations / casts — no FLOPs beyond /255 fold)
# ----------------------------------------------------------------------------

def host_prep(inputs, states, masks, action, conv1_w, conv1_b, conv2_w,
              conv2_b, conv3_w, conv3_b, fc_w, fc_b, w_ih, w_hh, b_ih, b_hh,
              actor_w, actor_b, critic_w, critic_b):
    f32 = np.float32
    inputs = np.asarray(inputs, f32)
    states = np.asarray(states, f32)
    masks = np.asarray(masks, f32)
    action = np.asarray(action)
    w1 = np.asarray(conv1_w, f32); b1 = np.asarray(conv1_b, f32)
    w2 = np.asarray(conv2_w, f32); b2 = np.asarray(conv2_b, f32)
    w3 = np.asarray(conv3_w, f32); b3 = np.asarray(conv3_b, f32)
    fc_w = np.asarray(fc_w, f32); fc_b = np.asarray(fc_b, f32)
    w_ih = np.asarray(w_ih, f32); w_hh = np.asarray(w_hh, f32)
    b_ih = np.asarray(b_ih, f32); b_hh = np.asarray(b_hh, f32)
    actor_w = np.asarray(actor_w, f32); actor_b = np.asarray(actor_b, f32)
    critic_w = np.asarray(critic_w, f32)
    critic_b = np.asarray(critic_b, f32)

    shared = {}
    # conv1 stationaries: S1[(g,rx)][p=(ry,c,dy,dx), (g',o)]
    w1s = w1 / 255.0
    S1 = np.zeros((4, 2, 128, 128), f32)
    for g in range(4):
        for rx in range(2):
            for ry in range(2):
                for c in range(C):
                    for dy in range(4):
                        for dx in range(4):
                            p = ry * 64 + c * 16 + dy * 4 + dx
                            S1[g, rx, p, g * 32:(g + 1) * 32] = \
                                w1s[:, c, 4 * ry + dy, 4 * rx + dx]
    shared["s1"] = S1.reshape(8, 128, 128)

    W2 = np.zeros((2, 2, 128, 128), f32)
    for ry in range(2):
        for rx in range(2):
            for g in range(4):
                dy2, dx2 = g // 2, g % 2
                for c in range(32):
                    col = w2[:, c, 2 * ry + dy2, 2 * rx + dx2]
                    W2[ry, rx, g * 32 + c, 0:64] = col
                    W2[ry, rx, g * 32 + c, 64:128] = col
    shared["w2"] = W2.reshape(4, 128, 128)

    W3A = np.zeros((3, 128, 128), f32)
    W3B = np.zeros((3, 64, 128), f32)
    for kx in range(3):
        for kyg in range(2):
            for c in range(64):
                for g4 in range(4):
                    W3A[kx, kyg * 64 + c, g4 * 32:(g4 + 1) * 32] = \
                        w3[:, c, kyg, kx]
        for c in range(64):
            for g4 in range(4):
                W3B[kx, c, g4 * 32:(g4 + 1) * 32] = w3[:, c, 2, kx]
    shared["w3a"] = W3A
    shared["w3b"] = W3B

    FCW = np.zeros((36, 4, 128, 128), f32)
    for j in range(36):
        for g in range(4):
            for c in range(32):
                FCW[j, :, g * 32 + c, :] = \
                    fc_w[:, c * 144 + 4 * j + g].reshape(4, 128)
    shared["fcw"] = FCW.reshape(144, 128, 128)

    GIW = np.zeros((4, 12, 128, 128), f32)
    WHH = np.zeros((4, 12, 128, 128), f32)
    for hc in range(4):
        for gc in range(12):
            GIW[hc, gc] = w_ih[gc * 128:(gc + 1) * 128,
                               hc * 128:(hc + 1) * 128].T
            WHH[hc, gc] = w_hh[gc * 128:(gc + 1) * 128,
                               hc * 128:(hc + 1) * 128].T
    shared["giw"] = GIW.reshape(48, 128, 128)
    shared["whh"] = WHH.reshape(48, 128, 128)

    ACm = np.concatenate([actor_w, critic_w], axis=0)       # [7, 512]
    ACW = np.zeros((4, 128, 7), f32)
    for hc in range(4):
        ACW[hc] = ACm[:, hc * 128:(hc + 1) * 128].T
    shared["acw"] = ACW

    shared["b1"] = np.tile(b1, 4).reshape(128, 1)
    shared["b2"] = np.tile(b2, 2).reshape(128, 1)
    shared["b3"] = np.tile(b3, 4).reshape(128, 1)
    shared["fcb"] = fc_b.reshape(4, 128).T.copy()
    gib = b_ih + np.concatenate([b_hh[:1024], np.zeros(512, f32)])
    shared["gib"] = gib.reshape(12, 128).T.copy()
    bhhn = b_hh[1024:].reshape(4, 128).T                     # [128, gc']
    shared["bhhn"] = np.repeat(bhhn, EPC, axis=1).copy()     # [(gc',e)]
    acb = np.concatenate([actor_b, critic_b]).astype(f32)
    shared["acb"] = np.tile(acb, (IPC, 1))

    for k in ("s1", "w2", "w3a", "w3b", "fcw", "giw", "whh"):
        shared[k] = shared[k].astype(BF16)

    # per-core tensors
    x_all = inputs.reshape(T, N, C, HW, HW)
    m_all = masks.reshape(T, N)
    a_all = np.asarray(action).reshape(T, N)
    in_maps = []
    # vectorized phase-plane build for all images at once:
    # P[(t,n), p=(ry,c,dy,dx), Y, X] = x_all[t, n, c, 4Y+4ry+dy, 4X+dx]
    xb = x_all.reshape(T * N, C, 31, 4, 31, 4)     # [i, c, Yr, dy, Xr, dx]
    P_all = np.zeros((T * N, 2, C, 4, 4, 30, 31), np.float32)
    for ry in range(2):
        P_all[:, ry] = xb[:, :, ry:ry + 30].transpose(
            0, 1, 3, 5, 2, 4)                       # [i, c, dy, dx, Y, X]
    P_all = P_all.reshape(T * N, 128, 930).astype(BF16)

    for k in range(NCORES):
        envs = [EPC * k + e for e in range(EPC)]
        idx = [t * N + n for t in range(T) for n in envs]
        m = dict(shared)
        m["p2"] = np.ascontiguousarray(P_all[idx])
        h0 = np.zeros((128, 4, EPC), np.float32)
        for e in range(EPC):
            h0[:, :, e] = states[envs[e]].reshape(4, 128).T
        m["h0"] = h0.reshape(128, 4 * EPC)
        mr = np.zeros((T, 4, EPC), np.float32)
        for e in range(EPC):
            mr[:, :, e] = m_all[:, envs[e]][:, None]
        m["mrep"] = np.broadcast_to(
            mr.reshape(1, T * 4 * EPC), (128, T * 4 * EPC)).copy()
        amk = np.zeros((IPC, A), np.float32)
        for t in range(T):
            for e in range(EPC):
                amk[t * EPC + e, int(a_all[t, envs[e]])] = 1.0
        m["am"] = amk
        in_maps.append(m)
    return in_maps


def assemble(results):
    value = np.zeros((T * N, 1), np.float32)
    alp = np.zeros((T * N,), np.float32)
    ent = np.zeros((T * N,), np.float32)
    st = np.zeros((N, H), np.float32)
    for k in range(NCORES):
        r = results[k]
        v = r["v"].reshape(T, EPC)
        a = r["alp"].reshape(T, EPC)
        e = r["ent"].reshape(T, EPC)
        for t in range(T):
            for ee in range(EPC):
                row = t * N + EPC * k + ee
                value[row, 0] = v[t, ee]
                alp[row] = a[t, ee]
                ent[row] = e[t, ee]
        stk = r["st"].reshape(128, 4, EPC)       # [p, hc, e]
        for e in range(EPC):
            st[EPC * k + e] = stk[:, :, e].T.reshape(H)
    return value, alp, ent, st


def kernel(**inputs):
    if "nc" not in _cache:
        _cache["nc"] = build_program(reps=1)
    nc = _cache["nc"]
    in_maps = host_prep(**inputs)
    res = run_bass_kernel_spmd(nc, in_maps, list(range(NCORES)))
    return assemble(res.results)


# revision 10
# speedup vs baseline: 2.4849x; 1.5820x over previous
"""Trainium2 Bass kernel for CNN+GRU actor-critic (T=32, N=16 envs, H=512).

Sharding: data-parallel over envs — each of the 8 cores processes 2 envs
x 32 timesteps = 64 images through the CNN trunk + fc + input-gate GEMM,
then runs the GRU recurrence locally for its 2 envs, then the actor/critic
heads. All parameters are replicated. No collectives; the host scatters
inputs and gathers outputs.

Layout strategy (per core):
  * conv1 (8x8 s4) consumes host-prepared "phase planes"
      P2[p=(ry,c,dy,dx), Y, X] = img[c, 4(Y+ry)+dy, 4X+dx]
    so the 256-term contraction becomes 2 accumulating K=128 matmuls (rx).
    The stationary is padded to M=128 output columns (g',o) so conv1's
    output lands directly in conv2's phase layout [(dy2,dx2,o), Y2, X2].
  * conv2 (4x4 s2): 4 accumulating K=128 matmuls over (ry,rx); stationary
    columns duplicated (kyg,o) so eviction produces the row-shifted pair
    QQ[(kyg,c), Yq, X] = relu(conv2)[c, Yq+kyg, X] that conv3 needs.
  * conv3 (3x3 s1): ky in {0,1} packed on partitions (K=128), ky=2 as a
    K=64 matmul; stationary columns duplicated (g4,o) so eviction writes
    DUP[(g4,c), yxq, img] = relu(conv3)[c, 4*yxq+g4], which is exactly the
    fc moving operand for k-chunks of 128 = (4 spatial positions x 32 ch).
  * fc / gi: plain chunked GEMMs, images batched in the moving free dim.
  * GRU scan, "form S": stationary = w_hh.T chunks (48 LDW/step, bf16 FWL),
    moving = h.T [128,2]; gates land on partitions -> cheap pointwise.
  * heads: lhsT = GRU outputs [128, 64 imgs], moving = [actor;critic].T
    [128,7] fp32; log-softmax / entropy / gather pointwise on [64,7].

Matmul inputs are bf16 (PSUM accumulation fp32); scan pointwise, GI and
heads are fp32. Validated vs the jax reference at rel err ~1e-3 (value),
~5e-3 (states_out, which has ~1e-3 scale).
"""
import numpy as np
import ml_dtypes

import concourse.bass as bass
import concourse.tile as tile
from concourse import bacc, mybir
from concourse.bass_utils import run_bass_kernel_spmd

T, N, C, HW, A, H = 32, 16, 4, 124, 6, 512
NCORES = 8
EPC = N // NCORES            # 2 envs per core
IPC = T * EPC                # 64 images per core
BF16 = ml_dtypes.bfloat16

F32 = mybir.dt.float32
BF = mybir.dt.bfloat16
AF = mybir.ActivationFunctionType
OP = mybir.AluOpType
AX = mybir.AxisListType

_cache = {}


# ----------------------------------------------------------------------------
# device program
# ----------------------------------------------------------------------------

def build_program(reps: int = 1, n_imgs: int = IPC, do_fc: bool = True, do_scan: bool = True, do_heads: bool = True):
    nc = bacc.Bacc("TRN2", target_bir_lowering=False, debug=False)

    di = {}

    def inp(name, shape, dt):
        di[name] = nc.dram_tensor(name, list(shape), dt, kind="ExternalInput")
        return di[name]

    def outp(name, shape, dt):
        di[name] = nc.dram_tensor(name, list(shape), dt, kind="ExternalOutput")
        return di[name]

    d_p2 = inp("p2", [IPC, 128, 930], BF)          # (30*31 free)
    d_s1 = inp("s1", [8, 128, 128], BF)            # (g*2+rx)
    d_w2 = inp("w2", [4, 128, 128], BF)            # (ry*2+rx)
    d_w3a = inp("w3a", [3, 128, 128], BF)          # kx
    d_w3b = inp("w3b", [3, 64, 128], BF)           # kx
    d_fcw = inp("fcw", [144, 128, 128], BF)        # (j*4+mc)
    d_giw = inp("giw", [48, 128, 128], BF)         # (hc*12+gc)
    d_whh = inp("whh", [48, 128, 128], BF)         # (hc*12+gc)
    d_acw = inp("acw", [4, 128, 7], F32)           # hc
    d_b1 = inp("b1", [128, 1], F32)
    d_b2 = inp("b2", [128, 1], F32)
    d_b3 = inp("b3", [128, 1], F32)
    d_fcb = inp("fcb", [128, 4], F32)
    d_gib = inp("gib", [128, 12], F32)
    d_bhhn = inp("bhhn", [128, 4 * EPC], F32)      # [(gc',e)] rep, gc'=0..3
    d_mrep = inp("mrep", [128, T * 4 * EPC], F32)  # [(t,hc,e)] rep over p,hc
    d_h0 = inp("h0", [128, 4 * EPC], F32)          # [(hc,e)]
    d_am = inp("am", [IPC, A], F32)                # one-hot(action)
    d_acb = inp("acb", [IPC, 7], F32)              # [actor_b, critic_b] rep

    d_v = outp("v", [IPC, 1], F32)
    d_alp = outp("alp", [IPC, 1], F32)
    d_ent = outp("ent", [IPC, 1], F32)
    d_st = outp("st", [128, 4 * EPC], F32)

    with tile.TileContext(nc) as tc:
        from contextlib import ExitStack
        with ExitStack() as ctx:
            cpool = ctx.enter_context(tc.tile_pool(name="consts", bufs=1))
            work = ctx.enter_context(tc.tile_pool(name="work", bufs=4))
            small = ctx.enter_context(tc.tile_pool(name="small", bufs=2))
            persist = ctx.enter_context(tc.tile_pool(name="persist", bufs=1))
            pscv = ctx.enter_context(
                tc.tile_pool(name="pscv", bufs=2, space="PSUM"))
            pssm = ctx.enter_context(
                tc.tile_pool(name="pssm", bufs=1, space="PSUM"))
            pshd = ctx.enter_context(
                tc.tile_pool(name="pshd", bufs=1, space="PSUM"))

            # ---- load constants ----
            def cload(dram, shape, dt):
                t = cpool.tile(list(shape), dt, tag=dram.name)
                if len(dram.shape) == 3:
                    a, p, m = dram.shape
                    nc.sync.dma_start(
                        t[:].rearrange("p (a m) -> p a m", a=a),
                        dram.ap().rearrange("a p m -> p a m"))
                else:
                    nc.sync.dma_start(t[:], dram[:])
                return t

            s1 = cload(d_s1, [128, 8 * 128], BF)
            w2 = cload(d_w2, [128, 4 * 128], BF)
            w3a = cload(d_w3a, [128, 3 * 128], BF)
            w3b = cload(d_w3b, [64, 3 * 128], BF)
            fcw = cload(d_fcw, [128, 144 * 128], BF)
            giw = cload(d_giw, [128, 48 * 128], BF)
            whh = cload(d_whh, [128, 48 * 128], BF)
            acw = cload(d_acw, [128, 4 * 7], F32)
            b1 = cload(d_b1, [128, 1], F32)
            b2 = cload(d_b2, [128, 1], F32)
            b3 = cload(d_b3, [128, 1], F32)
            fcb = cload(d_fcb, [128, 4], F32)
            gib = cload(d_gib, [128, 12], F32)
            bhhn = cload(d_bhhn, [128, 4 * EPC], F32)
            mrep = cload(d_mrep, [128, T * 4 * EPC], F32)
            h0 = cload(d_h0, [128, 4 * EPC], F32)
            am = cload(d_am, [IPC, A], F32)
            acb = cload(d_acb, [IPC, 7], F32)

            def body(_iv=None):
                E = EPC
                DUP = persist.tile([128, 36 * IPC], BF, tag="dup")
                FC = persist.tile([128, 4 * IPC], BF, tag="fc")
                GI = persist.tile([128, 12 * IPC], F32, tag="gi")
                OUTS = persist.tile([128, 4 * T * E], F32, tag="outs")
                OUTSv = OUTS[:].rearrange("p (hc t e) -> p hc t e", hc=4, t=T)

                # ---------------- CNN trunk, per image ----------------
                for i in range(n_imgs):
                    p2 = work.tile([128, 930], BF, tag="p2")
                    nc.sync.dma_start(p2[:], d_p2[i])
                    p2v = p2[:].rearrange("p (y x) -> p y x", y=30)

                    ps2 = pscv.tile([128, 225], F32, tag="c1")
                    k = 0
                    for g in range(4):
                        dy2, dx2 = g // 2, g % 2
                        for rx in range(2):
                            rhs = p2v[:, dy2:30:2, dx2 + rx:31:2][:, :15, :15]
                            nc.tensor.matmul(
                                ps2[:], s1[:, bass.ts(g * 2 + rx, 128)], rhs,
                                start=(k == 0), stop=(k == 7))
                            k += 1
                    q = work.tile([128, 225], BF, tag="q")
                    nc.scalar.activation(q[:], ps2[:], AF.Relu, bias=b1[:])
                    qv = q[:].rearrange("p (y x) -> p y x", y=15)

                    ps3 = pscv.tile([128, 196], F32, tag="c2")
                    k = 0
                    for ry in range(2):
                        for rx in range(2):
                            rhs = qv[:, ry:ry + 14, rx:rx + 14]
                            nc.tensor.matmul(
                                ps3[:], w2[:, bass.ts(ry * 2 + rx, 128)], rhs,
                                start=(k == 0), stop=(k == 3))
                            k += 1
                    ps3v = ps3[:].rearrange("p (y x) -> p y x", y=14)
                    qq = work.tile([128, 196], BF, tag="qq")
                    qqv = qq[:].rearrange("p (y x) -> p y x", y=14)
                    nc.vector.tensor_scalar(qqv[0:64], ps3v[0:64],
                                            b2[0:64], 0.0,
                                            op0=OP.add, op1=OP.max)
                    nc.vector.tensor_scalar(qqv[64:128, 0:13],
                                            ps3v[64:128, 1:14],
                                            b2[64:128], 0.0,
                                            op0=OP.add, op1=OP.max)

                    ps4 = pscv.tile([128, 144], F32, tag="c3")
                    for kx in range(3):
                        nc.tensor.matmul(
                            ps4[:], w3a[:, bass.ts(kx, 128)],
                            qqv[:, 0:12, kx:kx + 12],
                            start=(kx == 0), stop=False)
                    for kx in range(3):
                        nc.tensor.matmul(
                            ps4[:], w3b[:, bass.ts(kx, 128)],
                            qqv[0:64, 2:14, kx:kx + 12],
                            start=False, stop=(kx == 2))
                    dupv = DUP[:].rearrange("p (j i) -> p j i", j=36)
                    ps4v = ps4[:].rearrange("p (yx) -> p yx")
                    for g4 in range(4):
                        sl = slice(g4 * 32, (g4 + 1) * 32)
                        nc.scalar.activation(
                            dupv[sl, :, i],
                            ps4[sl, g4:144:4],
                            AF.Relu, bias=b3[sl])

                # ---------------- fc ----------------
                if not do_fc:
                    nc.sync.dma_start(d_v[:], am[:, 0:1])
                    nc.sync.dma_start(d_alp[:], am[:, 0:1])
                    nc.sync.dma_start(d_ent[:], am[:, 0:1])
                    nc.sync.dma_start(d_st[:], mrep[:, 0:4 * EPC])
                    return
                dupv = DUP[:].rearrange("p (j i) -> p j i", j=36)
                for mc in range(4):
                    psf = pssm.tile([128, IPC], F32, tag="sm")
                    for j in range(36):
                        nc.tensor.matmul(
                            psf[:], fcw[:, bass.ts(j * 4 + mc, 128)],
                            dupv[:, j, :],
                            start=(j == 0), stop=(j == 35))
                    nc.scalar.activation(FC[:, bass.ts(mc, IPC)], psf[:],
                                         AF.Relu, bias=fcb[:, mc:mc + 1])

                # ---------------- gi ----------------
                for gc in range(12):
                    psg = pssm.tile([128, IPC], F32, tag="sm")
                    for hc in range(4):
                        nc.tensor.matmul(
                            psg[:], giw[:, bass.ts(hc * 12 + gc, 128)],
                            FC[:, bass.ts(hc, IPC)],
                            start=(hc == 0), stop=(hc == 3))
                    nc.scalar.activation(GI[:, bass.ts(gc, IPC)], psg[:],
                                         AF.Identity, bias=gib[:, gc:gc + 1])
                GIv = GI[:].rearrange("p (gc t e) -> p gc t e", gc=12, t=T)

                # ---------------- GRU scan ----------------
                if not do_scan:
                    nc.sync.dma_start(d_v[:], am[:, 0:1])
                    nc.sync.dma_start(d_alp[:], am[:, 0:1])
                    nc.sync.dma_start(d_ent[:], am[:, 0:1])
                    nc.sync.dma_start(d_st[:], mrep[:, 0:4 * EPC])
                    return
                mrv = mrep[:].rearrange("p (t he) -> p t he", t=T)
                for t in range(T):
                    hprev = h0[:] if t == 0 else OUTSv[:, :, t - 1, :]
                    hm = small.tile([128, 4 * E], F32, tag="hm")
                    hmv = hm[:].rearrange("p (hc e) -> p hc e", hc=4)
                    nc.vector.tensor_tensor(hm[:], hprev, mrv[:, t, :],
                                            op=OP.mult)
                    hmb = small.tile([128, 4 * E], BF, tag="hmb")
                    nc.vector.tensor_copy(hmb[:], hm[:])
                    hmbv = hmb[:].rearrange("p (hc e) -> p hc e", hc=4)

                    pss = pssm.tile([128, 12 * E], F32, tag="sm")
                    for gc in range(12):
                        for hc in range(4):
                            nc.tensor.matmul(
                                pss[:, bass.ts(gc, E)],
                                whh[:, bass.ts(hc * 12 + gc, 128)],
                                hmbv[:, hc, :],
                                start=(hc == 0), stop=(hc == 3))

                    arz = small.tile([128, 8 * E], F32, tag="arz")
                    nc.vector.tensor_tensor(
                        arz[:], pss[:, 0:8 * E],
                        GIv[:, 0:8, t, :], op=OP.add)
                    rz = small.tile([128, 8 * E], F32, tag="rz")
                    nc.scalar.activation(rz[:], arz[:], AF.Sigmoid)
                    bn = small.tile([128, 4 * E], F32, tag="bn")
                    nc.vector.tensor_tensor(
                        bn[:], pss[:, 8 * E:12 * E], bhhn[:], op=OP.add)
                    t1 = small.tile([128, 4 * E], F32, tag="t1")
                    nc.vector.tensor_tensor(t1[:], rz[:, 0:4 * E], bn[:],
                                            op=OP.mult)
                    t2 = small.tile([128, 4 * E], F32, tag="t2")
                    nc.vector.tensor_tensor(t2[:], t1[:], GIv[:, 8:12, t, :],
                                            op=OP.add)
                    nn = small.tile([128, 4 * E], F32, tag="nn")
                    nc.scalar.activation(nn[:], t2[:], AF.Tanh)
                    dd = small.tile([128, 4 * E], F32, tag="dd")
                    nc.vector.tensor_tensor(dd[:], hm[:], nn[:], op=OP.subtract)
                    e1 = small.tile([128, 4 * E], F32, tag="e1")
                    nc.vector.tensor_tensor(e1[:], rz[:, 4 * E:8 * E], dd[:],
                                            op=OP.mult)
                    nc.vector.tensor_tensor(OUTSv[:, :, t, :], nn[:], e1[:],
                                            op=OP.add)

                # ---------------- heads ----------------
                if not do_heads:
                    nc.sync.dma_start(d_v[:], am[:, 0:1])
                    nc.sync.dma_start(d_alp[:], am[:, 0:1])
                    nc.sync.dma_start(d_ent[:], am[:, 0:1])
                    nc.sync.dma_start(
                        d_st.ap().rearrange("p (hc e) -> p hc e", hc=4),
                        OUTSv[:, :, T - 1, :])
                    return
                psl = pshd.tile([IPC, 7], F32, tag="hd")
                for hc in range(4):
                    nc.tensor.matmul(
                        psl[:], OUTSv[:, hc, :, :], acw[:, bass.ts(hc, 7)],
                        start=(hc == 0), stop=(hc == 3))
                lb = small.tile([IPC, 7], F32, tag="lb")
                nc.vector.tensor_tensor(lb[:], psl[:], acb[:], op=OP.add)
                mx = small.tile([IPC, 1], F32, tag="mx")
                nc.vector.tensor_reduce(mx[:], lb[:, 0:6], axis=AX.X,
                                        op=OP.max)
                sh = small.tile([IPC, 6], F32, tag="sh")
                nc.vector.tensor_scalar(sh[:], lb[:, 0:6], mx[:], None,
                                        op0=OP.subtract)
                ee = small.tile([IPC, 6], F32, tag="ee")
                ss = small.tile([IPC, 1], F32, tag="ss")
                nc.scalar.activation(ee[:], sh[:], AF.Exp, accum_out=ss[:])
                ls = small.tile([IPC, 1], F32, tag="ls")
                nc.scalar.activation(ls[:], ss[:], AF.Ln)
                logp = small.tile([IPC, 6], F32, tag="logp")
                nc.vector.tensor_scalar(logp[:], sh[:], ls[:], None,
                                        op0=OP.subtract)
                ta = small.tile([IPC, 6], F32, tag="ta")
                nc.vector.tensor_tensor(ta[:], logp[:], am[:], op=OP.mult)
                alp_t = small.tile([IPC, 1], F32, tag="alp")
                nc.vector.tensor_reduce(alp_t[:], ta[:], axis=AX.X, op=OP.add)
                rs = small.tile([IPC, 1], F32, tag="rs")
                nc.vector.reciprocal(rs[:], ss[:])
                pp = small.tile([IPC, 6], F32, tag="pp")
                nc.vector.tensor_scalar(pp[:], ee[:], rs[:], None,
                                        op0=OP.mult)
                tb = small.tile([IPC, 6], F32, tag="tb")
                nc.vector.tensor_tensor(tb[:], pp[:], logp[:], op=OP.mult)
                tbs = small.tile([IPC, 1], F32, tag="tbs")
                nc.vector.tensor_reduce(tbs[:], tb[:], axis=AX.X, op=OP.add)
                ent_t = small.tile([IPC, 1], F32, tag="ent")
                nc.vector.tensor_scalar(ent_t[:], tbs[:], -1.0, None,
                                        op0=OP.mult)
                vv = small.tile([IPC, 1], F32, tag="vv")
                nc.vector.tensor_copy(vv[:], lb[:, 6:7])

                nc.sync.dma_start(d_v[:], vv[:])
                nc.sync.dma_start(d_alp[:], alp_t[:])
                nc.sync.dma_start(d_ent[:], ent_t[:])
                nc.sync.dma_start(
                    d_st.ap().rearrange("p (hc e) -> p hc e", hc=4),
                    OUTSv[:, :, T - 1, :])

            if reps == 1:
                body()
            else:
                with tc.For_i(0, reps, 1) as _i:
                    body(_i)

    nc.compile()
    return nc


# ----------------------------------------------------------------------------
# host-side data prep (pure permutations / casts — no FLOPs beyond /255 fold)
# ----------------------------------------------------------------------------

def host_prep(inputs, states, masks, action, conv1_w, conv1_b, conv2_w,
              conv2_b, conv3_w, conv3_b, fc_w, fc_b, w_ih, w_hh, b_ih, b_hh,
              actor_w, actor_b, critic_w, critic_b):
    f32 = np.float32
    inputs = np.asarray(inputs, f32)
    states = np.asarray(states, f32)
    masks = np.asarray(masks, f32)
    action = np.asarray(action)
    w1 = np.asarray(conv1_w, f32); b1 = np.asarray(conv1_b, f32)
    w2 = np.asarray(conv2_w, f32); b2 = np.asarray(conv2_b, f32)
    w3 = np.asarray(conv3_w, f32); b3 = np.asarray(conv3_b, f32)
    fc_w = np.asarray(fc_w, f32); fc_b = np.asarray(fc_b, f32)
    w_ih = np.asarray(w_ih, f32); w_hh = np.asarray(w_hh, f32)
    b_ih = np.asarray(b_ih, f32); b_hh = np.asarray(b_hh, f32)
    actor_w = np.asarray(actor_w, f32); actor_b = np.asarray(actor_b, f32)
    critic_w = np.asarray(critic_w, f32)
    critic_b = np.asarray(critic_b, f32)

    shared = {}
    # conv1 stationaries: S1[(g,rx)][p=(ry,c,dy,dx), (g',o)]
    w1s = w1 / 255.0
    S1 = np.zeros((4, 2, 128, 128), f32)
    for g in range(4):
        for rx in range(2):
            for ry in range(2):
                for c in range(C):
                    for dy in range(4):
                        for dx in range(4):
                            p = ry * 64 + c * 16 + dy * 4 + dx
                            S1[g, rx, p, g * 32:(g + 1) * 32] = \
                                w1s[:, c, 4 * ry + dy, 4 * rx + dx]
    shared["s1"] = S1.reshape(8, 128, 128)

    W2 = np.zeros((2, 2, 128, 128), f32)
    for ry in range(2):
        for rx in range(2):
            for g in range(4):
                dy2, dx2 = g // 2, g % 2
                for c in range(32):
                    col = w2[:, c, 2 * ry + dy2, 2 * rx + dx2]
                    W2[ry, rx, g * 32 + c, 0:64] = col
                    W2[ry, rx, g * 32 + c, 64:128] = col
    shared["w2"] = W2.reshape(4, 128, 128)

    W3A = np.zeros((3, 128, 128), f32)
    W3B = np.zeros((3, 64, 128), f32)
    for kx in range(3):
        for kyg in range(2):
            for c in range(64):
                for g4 in range(4):
                    W3A[kx, kyg * 64 + c, g4 * 32:(g4 + 1) * 32] = \
                        w3[:, c, kyg, kx]
        for c in range(64):
            for g4 in range(4):
                W3B[kx, c, g4 * 32:(g4 + 1) * 32] = w3[:, c, 2, kx]
    shared["w3a"] = W3A
    shared["w3b"] = W3B

    FCW = np.zeros((36, 4, 128, 128), f32)
    for j in range(36):
        for g in range(4):
            for c in range(32):
                FCW[j, :, g * 32 + c, :] = \
                    fc_w[:, c * 144 + 4 * j + g].reshape(4, 128)
    shared["fcw"] = FCW.reshape(144, 128, 128)

    GIW = np.zeros((4, 12, 128, 128), f32)
    WHH = np.zeros((4, 12, 128, 128), f32)
    for hc in range(4):
        for gc in range(12):
            GIW[hc, gc] = w_ih[gc * 128:(gc + 1) * 128,
                               hc * 128:(hc + 1) * 128].T
            WHH[hc, gc] = w_hh[gc * 128:(gc + 1) * 128,
                               hc * 128:(hc + 1) * 128].T
    shared["giw"] = GIW.reshape(48, 128, 128)
    shared["whh"] = WHH.reshape(48, 128, 128)

    ACm = np.concatenate([actor_w, critic_w], axis=0)       # [7, 512]
    ACW = np.zeros((4, 128, 7), f32)
    for hc in range(4):
        ACW[hc] = ACm[:, hc * 128:(hc + 1) * 128].T
    shared["acw"] = ACW

    shared["b1"] = np.tile(b1, 4).reshape(128, 1)
    shared["b2"] = np.tile(b2, 2).reshape(128, 1)
    shared["b3"] = np.tile(b3, 4).reshape(128, 1)
    shared["fcb"] = fc_b.reshape(4, 128).T.copy()
    gib = b_ih + np.concatenate([b_hh[:1024], np.zeros(512, f32)])
    shared["gib"] = gib.reshape(12, 128).T.copy()
    bhhn = b_hh[1024:].reshape(4, 128).T                     # [128, gc']
    shared["bhhn"] = np.repeat(bhhn, EPC, axis=1).copy()     # [(gc',e)]
    acb = np.concatenate([actor_b, critic_b]).astype(f32)
    shared["acb"] = np.tile(acb, (IPC, 1))

    for k in ("s1", "w2", "w3a", "w3b", "fcw", "giw", "whh"):
        shared[k] = shared[k].astype(BF16)

    # per-core tensors
    x_all = inputs.reshape(T, N, C, HW, HW)
    m_all = masks.reshape(T, N)
    a_all = np.asarray(action).reshape(T, N)
    in_maps = []
    # vectorized phase-plane build for all images at once:
    # P[(t,n), p=(ry,c,dy,dx), Y, X] = x_all[t, n, c, 4Y+4ry+dy, 4X+dx]
    xb = x_all.reshape(T * N, C, 31, 4, 31, 4)     # [i, c, Yr, dy, Xr, dx]
    P_all = np.zeros((T * N, 2, C, 4, 4, 30, 31), np.float32)
    for ry in range(2):
        P_all[:, ry] = xb[:, :, ry:ry + 30].transpose(
            0, 1, 3, 5, 2, 4)                       # [i, c, dy, dx, Y, X]
    P_all = P_all.reshape(T * N, 128, 930).astype(BF16)

    for k in range(NCORES):
        envs = [EPC * k + e for e in range(EPC)]
        idx = [t * N + n for t in range(T) for n in envs]
        m = dict(shared)
        m["p2"] = np.ascontiguousarray(P_all[idx])
        h0 = np.zeros((128, 4, EPC), np.float32)
        for e in range(EPC):
            h0[:, :, e] = states[envs[e]].reshape(4, 128).T
        m["h0"] = h0.reshape(128, 4 * EPC)
        mr = np.zeros((T, 4, EPC), np.float32)
        for e in range(EPC):
            mr[:, :, e] = m_all[:, envs[e]][:, None]
        m["mrep"] = np.broadcast_to(
            mr.reshape(1, T * 4 * EPC), (128, T * 4 * EPC)).copy()
        amk = np.zeros((IPC, A), np.float32)
        for t in range(T):
            for e in range(EPC):
                amk[t * EPC + e, int(a_all[t, envs[e]])] = 1.0
        m["am"] = amk
        in_maps.append(m)
    return in_maps


def assemble(results):
    value = np.zeros((T * N, 1), np.float32)
    alp = np.zeros((T * N,), np.float32)
    ent = np.zeros((T * N,), np.float32)
    st = np.zeros((N, H), np.float32)
    for k in range(NCORES):
        r = results[k]
        v = r["v"].reshape(T, EPC)
        a = r["alp"].reshape(T, EPC)
        e = r["ent"].reshape(T, EPC)
        for t in range(T):
            for ee in range(EPC):
                row = t * N + EPC * k + ee
                value[row, 0] = v[t, ee]
                alp[row] = a[t, ee]
                ent[row] = e[t, ee]
        stk = r["st"].reshape(128, 4, EPC)       # [p, hc, e]
        for e in range(EPC):
            st[EPC * k + e] = stk[:, :, e].T.reshape(H)
    return value, alp, ent, st


def kernel(**inputs):
    if "nc" not in _cache:
        _cache["nc"] = build_program(reps=1)
    nc = _cache["nc"]
    in_maps = host_prep(**inputs)
    res = run_bass_kernel_spmd(nc, in_maps, list(range(NCORES)))
    return assemble(res.results)


# revision 12
# speedup vs baseline: 2.6374x; 1.0614x over previous
"""Trainium2 Bass kernel for CNN+GRU actor-critic (T=32, N=16 envs, H=512).

Sharding: data-parallel over envs — each of the 8 cores processes 2 envs
x 32 timesteps = 64 images through the CNN trunk + fc + input-gate GEMM,
then runs the GRU recurrence locally for its 2 envs, then the actor/critic
heads. All parameters are replicated. No collectives; the host scatters
inputs and gathers outputs.

Layout strategy (per core):
  * conv1 (8x8 s4) consumes host-prepared "phase planes"
      P2[p=(ry,c,dy,dx), Y, X] = img[c, 4(Y+ry)+dy, 4X+dx]
    so the 256-term contraction becomes 2 accumulating K=128 matmuls (rx).
    The stationary is padded to M=128 output columns (g',o) so conv1's
    output lands directly in conv2's phase layout [(dy2,dx2,o), Y2, X2].
  * conv2 (4x4 s2): 4 accumulating K=128 matmuls over (ry,rx); stationary
    columns duplicated (kyg,o) so eviction produces the row-shifted pair
    QQ[(kyg,c), Yq, X] = relu(conv2)[c, Yq+kyg, X] that conv3 needs.
  * conv3 (3x3 s1): ky in {0,1} packed on partitions (K=128), ky=2 as a
    K=64 matmul; stationary columns duplicated (g4,o) so eviction writes
    DUP[(g4,c), yxq, img] = relu(conv3)[c, 4*yxq+g4], which is exactly the
    fc moving operand for k-chunks of 128 = (4 spatial positions x 32 ch).
  * fc / gi: plain chunked GEMMs, images batched in the moving free dim.
  * GRU scan, "form S": stationary = w_hh.T chunks (48 LDW/step, bf16 FWL),
    moving = h.T [128,2]; gates land on partitions -> cheap pointwise.
  * heads: lhsT = GRU outputs [128, 64 imgs], moving = [actor;critic].T
    [128,7] fp32; log-softmax / entropy / gather pointwise on [64,7].

Matmul inputs are bf16 (PSUM accumulation fp32); scan pointwise, GI and
heads are fp32. Validated vs the jax reference at rel err ~1e-3 (value),
~5e-3 (states_out, which has ~1e-3 scale).
"""
import numpy as np
import ml_dtypes

import concourse.bass as bass
import concourse.tile as tile
from concourse import bacc, mybir
from concourse.bass_utils import run_bass_kernel_spmd

T, N, C, HW, A, H = 32, 16, 4, 124, 6, 512
NCORES = 8
EPC = N // NCORES            # 2 envs per core
IPC = T * EPC                # 64 images per core
BF16 = ml_dtypes.bfloat16

F32 = mybir.dt.float32
BF = mybir.dt.bfloat16
AF = mybir.ActivationFunctionType
OP = mybir.AluOpType
AX = mybir.AxisListType

_cache = {}


# ----------------------------------------------------------------------------
# device program
# ----------------------------------------------------------------------------

def build_program(reps: int = 1, n_imgs: int = IPC, do_fc: bool = True, do_scan: bool = True, do_heads: bool = True, masks_ones: bool = False):
    nc = bacc.Bacc("TRN2", target_bir_lowering=False, debug=False)

    di = {}

    def inp(name, shape, dt):
        di[name] = nc.dram_tensor(name, list(shape), dt, kind="ExternalInput")
        return di[name]

    def outp(name, shape, dt):
        di[name] = nc.dram_tensor(name, list(shape), dt, kind="ExternalOutput")
        return di[name]

    d_p2 = inp("p2", [IPC, 128, 930], BF)          # (30*31 free)
    d_s1 = inp("s1", [8, 128, 128], BF)            # (g*2+rx)
    d_w2 = inp("w2", [4, 128, 128], BF)            # (ry*2+rx)
    d_w3a = inp("w3a", [3, 128, 128], BF)          # kx
    d_w3b = inp("w3b", [3, 64, 128], BF)           # kx
    d_fcw = inp("fcw", [144, 128, 128], BF)        # (j*4+mc)
    d_giw = inp("giw", [48, 128, 128], BF)         # (hc*12+gc)
    d_whh = inp("whh", [48, 128, 128], BF)         # (hc*12+gc)
    d_acw = inp("acw", [4, 128, 7], F32)           # hc
    d_b1 = inp("b1", [128, 1], F32)
    d_b2 = inp("b2", [128, 1], F32)
    d_b3 = inp("b3", [128, 1], F32)
    d_fcb = inp("fcb", [128, 4], F32)
    d_gib = inp("gib", [128, 12], F32)
    d_bhhn = inp("bhhn", [128, 4 * EPC], F32)      # [(gc',e)] rep, gc'=0..3
    d_mrep = inp("mrep", [128, T * 4 * EPC], F32)  # [(t,hc,e)] rep over p,hc
    d_h0 = inp("h0", [128, 4 * EPC], F32)          # [(hc,e)]
    d_am = inp("am", [IPC, A], F32)                # one-hot(action)
    d_acb = inp("acb", [IPC, 7], F32)              # [actor_b, critic_b] rep

    d_v = outp("v", [IPC, 1], F32)
    d_alp = outp("alp", [IPC, 1], F32)
    d_ent = outp("ent", [IPC, 1], F32)
    d_st = outp("st", [128, 4 * EPC], F32)

    with tile.TileContext(nc) as tc:
        from contextlib import ExitStack
        with ExitStack() as ctx:
            cpool = ctx.enter_context(tc.tile_pool(name="consts", bufs=1))
            work = ctx.enter_context(tc.tile_pool(name="work", bufs=4))
            small = ctx.enter_context(tc.tile_pool(name="small", bufs=2))
            persist = ctx.enter_context(tc.tile_pool(name="persist", bufs=1))
            pscv = ctx.enter_context(
                tc.tile_pool(name="pscv", bufs=2, space="PSUM"))
            pssm = ctx.enter_context(
                tc.tile_pool(name="pssm", bufs=1, space="PSUM"))
            pshd = ctx.enter_context(
                tc.tile_pool(name="pshd", bufs=1, space="PSUM"))

            # ---- load constants ----
            def cload(dram, shape, dt):
                t = cpool.tile(list(shape), dt, tag=dram.name)
                if len(dram.shape) == 3:
                    a, p, m = dram.shape
                    nc.sync.dma_start(
                        t[:].rearrange("p (a m) -> p a m", a=a),
                        dram.ap().rearrange("a p m -> p a m"))
                else:
                    nc.sync.dma_start(t[:], dram[:])
                return t

            s1 = cload(d_s1, [128, 8 * 128], BF)
            w2 = cload(d_w2, [128, 4 * 128], BF)
            w3a = cload(d_w3a, [128, 3 * 128], BF)
            w3b = cload(d_w3b, [64, 3 * 128], BF)
            fcw = cload(d_fcw, [128, 144 * 128], BF)
            giw = cload(d_giw, [128, 48 * 128], BF)
            whh = cload(d_whh, [128, 48 * 128], BF)
            acw = cload(d_acw, [128, 4 * 7], F32)
            b1 = cload(d_b1, [128, 1], F32)
            b2 = cload(d_b2, [128, 1], F32)
            b3 = cload(d_b3, [128, 1], F32)
            fcb = cload(d_fcb, [128, 4], F32)
            gib = cload(d_gib, [128, 12], F32)
            bhhn = cload(d_bhhn, [128, 4 * EPC], F32)
            mrep = cload(d_mrep, [128, T * 4 * EPC], F32)
            h0 = cload(d_h0, [128, 4 * EPC], F32)
            am = cload(d_am, [IPC, A], F32)
            acb = cload(d_acb, [IPC, 7], F32)

            def body(_iv=None):
                E = EPC
                DUP = persist.tile([128, 36 * IPC], BF, tag="dup")
                FC = persist.tile([128, 4 * IPC], BF, tag="fc")
                GI = persist.tile([128, 12 * IPC], F32, tag="gi")
                OUTS = persist.tile([128, 4 * T * E], F32, tag="outs")
                OUTSv = OUTS[:].rearrange("p (hc t e) -> p hc t e", hc=4, t=T)

                # ------------- CNN trunk, 2 images per matmul -------------
                P = 2
                dup_i = DUP[:].rearrange("p (j i) -> p i j", j=36)
                for ip in range(n_imgs // P):
                    i0 = ip * P
                    p2 = work.tile([128, P * 930], BF, tag="p2")
                    nc.sync.dma_start(
                        p2[:].rearrange("p (i f) -> p i f", i=P),
                        d_p2[i0:i0 + P].rearrange("i p f -> p i f"))
                    p2v = p2[:].rearrange("p (i y x) -> p i y x", i=P, y=30)

                    ps2 = pscv.tile([128, P * 225], F32, tag="c1")
                    k = 0
                    for g in range(4):
                        dy2, dx2 = g // 2, g % 2
                        for rx in range(2):
                            rhs = p2v[:, :, dy2:30:2,
                                      dx2 + rx:31:2][:, :, :15, :15]
                            nc.tensor.matmul(
                                ps2[:], s1[:, bass.ts(g * 2 + rx, 128)], rhs,
                                start=(k == 0), stop=(k == 7))
                            k += 1
                    q = work.tile([128, P * 225], BF, tag="q")
                    nc.scalar.activation(q[:], ps2[:], AF.Relu, bias=b1[:])
                    qv = q[:].rearrange("p (i y x) -> p i y x", i=P, y=15)

                    ps3 = pscv.tile([128, P * 196], F32, tag="c2")
                    k = 0
                    for ry in range(2):
                        for rx in range(2):
                            rhs = qv[:, :, ry:ry + 14, rx:rx + 14]
                            nc.tensor.matmul(
                                ps3[:], w2[:, bass.ts(ry * 2 + rx, 128)], rhs,
                                start=(k == 0), stop=(k == 3))
                            k += 1
                    ps3v = ps3[:].rearrange("p (i y x) -> p i y x", i=P, y=14)
                    qq = work.tile([128, P * 196], BF, tag="qq")
                    qqv = qq[:].rearrange("p (i y x) -> p i y x", i=P, y=14)
                    nc.vector.tensor_scalar(qqv[0:64], ps3v[0:64],
                                            b2[0:64], 0.0,
                                            op0=OP.add, op1=OP.max)
                    nc.vector.tensor_scalar(qqv[64:128, :, 0:13],
                                            ps3v[64:128, :, 1:14],
                                            b2[64:128], 0.0,
                                            op0=OP.add, op1=OP.max)

                    ps4 = pscv.tile([128, P * 144], F32, tag="c3")
                    for kx in range(3):
                        nc.tensor.matmul(
                            ps4[:], w3a[:, bass.ts(kx, 128)],
                            qqv[:, :, 0:12, kx:kx + 12],
                            start=(kx == 0), stop=False)
                    for kx in range(3):
                        nc.tensor.matmul(
                            ps4[:], w3b[:, bass.ts(kx, 128)],
                            qqv[0:64, :, 2:14, kx:kx + 12],
                            start=False, stop=(kx == 2))
                    ps4v = ps4[:].rearrange("p (i yx) -> p i yx", i=P)
                    for g4 in range(4):
                        sl = slice(g4 * 32, (g4 + 1) * 32)
                        nc.scalar.activation(
                            dup_i[sl, i0:i0 + P, :],
                            ps4v[sl, :, g4:144:4],
                            AF.Relu, bias=b3[sl])

                # ---------------- fc ----------------
                if not do_fc:
                    nc.sync.dma_start(d_v[:], am[:, 0:1])
                    nc.sync.dma_start(d_alp[:], am[:, 0:1])
                    nc.sync.dma_start(d_ent[:], am[:, 0:1])
                    nc.sync.dma_start(d_st[:], mrep[:, 0:4 * EPC])
                    return
                dupv = DUP[:].rearrange("p (j i) -> p j i", j=36)
                for mc in range(4):
                    psf = pssm.tile([128, IPC], F32, tag="sm")
                    for j in range(36):
                        nc.tensor.matmul(
                            psf[:], fcw[:, bass.ts(j * 4 + mc, 128)],
                            dupv[:, j, :],
                            start=(j == 0), stop=(j == 35))
                    nc.scalar.activation(FC[:, bass.ts(mc, IPC)], psf[:],
                                         AF.Relu, bias=fcb[:, mc:mc + 1])

                # ---------------- gi ----------------
                for gc in range(12):
                    psg = pssm.tile([128, IPC], F32, tag="sm")
                    for hc in range(4):
                        nc.tensor.matmul(
                            psg[:], giw[:, bass.ts(hc * 12 + gc, 128)],
                            FC[:, bass.ts(hc, IPC)],
                            start=(hc == 0), stop=(hc == 3))
                    nc.scalar.activation(GI[:, bass.ts(gc, IPC)], psg[:],
                                         AF.Identity, bias=gib[:, gc:gc + 1])
                GIv = GI[:].rearrange("p (gc t e) -> p gc t e", gc=12, t=T)

                # ---------------- GRU scan ----------------
                if not do_scan:
                    nc.sync.dma_start(d_v[:], am[:, 0:1])
                    nc.sync.dma_start(d_alp[:], am[:, 0:1])
                    nc.sync.dma_start(d_ent[:], am[:, 0:1])
                    nc.sync.dma_start(d_st[:], mrep[:, 0:4 * EPC])
                    return
                mrv = mrep[:].rearrange("p (t he) -> p t he", t=T)
                prev_zh = prev_m1 = None
                for t in range(T):
                    hprev = h0[:] if t == 0 else OUTSv[:, :, t - 1, :]
                    if not masks_ones:
                        hm = small.tile([128, 4 * E], F32, tag="hm")
                        nc.vector.tensor_tensor(hm[:], hprev, mrv[:, t, :],
                                                op=OP.mult)
                        hm_ap = hm[:]
                    else:
                        hm_ap = hprev
                    hmb = small.tile([128, 4 * E], BF, tag="hmb")
                    if masks_ones and prev_zh is not None:
                        nc.vector.tensor_tensor(hmb[:], prev_zh, prev_m1,
                                                op=OP.add)
                    else:
                        nc.vector.tensor_copy(hmb[:], hm_ap)
                    hmbv = hmb[:].rearrange("p (hc e) -> p hc e", hc=4)

                    pss = pssm.tile([128, 12 * E], F32, tag="sm")
                    for gc in range(12):
                        for hc in range(4):
                            nc.tensor.matmul(
                                pss[:, bass.ts(gc, E)],
                                whh[:, bass.ts(hc * 12 + gc, 128)],
                                hmbv[:, hc, :],
                                start=(hc == 0), stop=(hc == 3))

                    arz = small.tile([128, 8 * E], F32, tag="arz")
                    nc.vector.tensor_tensor(
                        arz[:], pss[:, 0:8 * E],
                        GIv[:, 0:8, t, :], op=OP.add)
                    rz = small.tile([128, 8 * E], F32, tag="rz")
                    nc.scalar.activation(rz[:], arz[:], AF.Sigmoid)
                    # h' = z*h + (1-z)*n, restructured so the bf16 state for
                    # the next step's matmuls is 2 ops past tanh.
                    zh = small.tile([128, 4 * E], F32, tag="zh")
                    nc.vector.tensor_tensor(zh[:], rz[:, 4 * E:8 * E], hm_ap,
                                            op=OP.mult)
                    zz = small.tile([128, 4 * E], F32, tag="zz")
                    nc.vector.tensor_scalar(zz[:], rz[:, 4 * E:8 * E],
                                            -1.0, 1.0, op0=OP.mult, op1=OP.add)
                    bn = small.tile([128, 4 * E], F32, tag="bn")
                    nc.vector.tensor_tensor(
                        bn[:], pss[:, 8 * E:12 * E], bhhn[:], op=OP.add)
                    t1 = small.tile([128, 4 * E], F32, tag="t1")
                    nc.vector.tensor_tensor(t1[:], rz[:, 0:4 * E], bn[:],
                                            op=OP.mult)
                    t2 = small.tile([128, 4 * E], F32, tag="t2")
                    nc.vector.tensor_tensor(t2[:], t1[:], GIv[:, 8:12, t, :],
                                            op=OP.add)
                    nn = small.tile([128, 4 * E], F32, tag="nn")
                    nc.scalar.activation(nn[:], t2[:], AF.Tanh)
                    m1 = small.tile([128, 4 * E], F32, tag="m1")
                    nc.vector.tensor_tensor(m1[:], zz[:], nn[:], op=OP.mult)
                    nc.gpsimd.tensor_tensor(OUTSv[:, :, t, :], zh[:], m1[:],
                                            op=OP.add)
                    prev_zh, prev_m1 = zh[:], m1[:]

                # ---------------- heads ----------------
                if not do_heads:
                    nc.sync.dma_start(d_v[:], am[:, 0:1])
                    nc.sync.dma_start(d_alp[:], am[:, 0:1])
                    nc.sync.dma_start(d_ent[:], am[:, 0:1])
                    nc.sync.dma_start(
                        d_st.ap().rearrange("p (hc e) -> p hc e", hc=4),
                        OUTSv[:, :, T - 1, :])
                    return
                psl = pshd.tile([IPC, 7], F32, tag="hd")
                for hc in range(4):
                    nc.tensor.matmul(
                        psl[:], OUTSv[:, hc, :, :], acw[:, bass.ts(hc, 7)],
                        start=(hc == 0), stop=(hc == 3))
                lb = small.tile([IPC, 7], F32, tag="lb")
                nc.vector.tensor_tensor(lb[:], psl[:], acb[:], op=OP.add)
                mx = small.tile([IPC, 1], F32, tag="mx")
                nc.vector.tensor_reduce(mx[:], lb[:, 0:6], axis=AX.X,
                                        op=OP.max)
                sh = small.tile([IPC, 6], F32, tag="sh")
                nc.vector.tensor_scalar(sh[:], lb[:, 0:6], mx[:], None,
                                        op0=OP.subtract)
                ee = small.tile([IPC, 6], F32, tag="ee")
                ss = small.tile([IPC, 1], F32, tag="ss")
                nc.scalar.activation(ee[:], sh[:], AF.Exp, accum_out=ss[:])
                ls = small.tile([IPC, 1], F32, tag="ls")
                nc.scalar.activation(ls[:], ss[:], AF.Ln)
                logp = small.tile([IPC, 6], F32, tag="logp")
                nc.vector.tensor_scalar(logp[:], sh[:], ls[:], None,
                                        op0=OP.subtract)
                ta = small.tile([IPC, 6], F32, tag="ta")
                nc.vector.tensor_tensor(ta[:], logp[:], am[:], op=OP.mult)
                alp_t = small.tile([IPC, 1], F32, tag="alp")
                nc.vector.tensor_reduce(alp_t[:], ta[:], axis=AX.X, op=OP.add)
                rs = small.tile([IPC, 1], F32, tag="rs")
                nc.vector.reciprocal(rs[:], ss[:])
                pp = small.tile([IPC, 6], F32, tag="pp")
                nc.vector.tensor_scalar(pp[:], ee[:], rs[:], None,
                                        op0=OP.mult)
                tb = small.tile([IPC, 6], F32, tag="tb")
                nc.vector.tensor_tensor(tb[:], pp[:], logp[:], op=OP.mult)
                tbs = small.tile([IPC, 1], F32, tag="tbs")
                nc.vector.tensor_reduce(tbs[:], tb[:], axis=AX.X, op=OP.add)
                ent_t = small.tile([IPC, 1], F32, tag="ent")
                nc.vector.tensor_scalar(ent_t[:], tbs[:], -1.0, None,
                                        op0=OP.mult)
                vv = small.tile([IPC, 1], F32, tag="vv")
                nc.vector.tensor_copy(vv[:], lb[:, 6:7])

                nc.sync.dma_start(d_v[:], vv[:])
                nc.sync.dma_start(d_alp[:], alp_t[:])
                nc.sync.dma_start(d_ent[:], ent_t[:])
                nc.sync.dma_start(
                    d_st.ap().rearrange("p (hc e) -> p hc e", hc=4),
                    OUTSv[:, :, T - 1, :])

            if reps == 1:
                body()
            else:
                with tc.For_i(0, reps, 1) as _i:
                    body(_i)

    nc.compile()
    return nc


# ----------------------------------------------------------------------------
# host-side data prep (pure permutations / casts — no FLOPs beyond /255 fold)
# ----------------------------------------------------------------------------

def host_prep(inputs, states, masks, action, conv1_w, conv1_b, conv2_w,
              conv2_b, conv3_w, conv3_b, fc_w, fc_b, w_ih, w_hh, b_ih, b_hh,
              actor_w, actor_b, critic_w, critic_b):
    f32 = np.float32
    inputs = np.asarray(inputs, f32)
    states = np.asarray(states, f32)
    masks = np.asarray(masks, f32)
    action = np.asarray(action)
    w1 = np.asarray(conv1_w, f32); b1 = np.asarray(conv1_b, f32)
    w2 = np.asarray(conv2_w, f32); b2 = np.asarray(conv2_b, f32)
    w3 = np.asarray(conv3_w, f32); b3 = np.asarray(conv3_b, f32)
    fc_w = np.asarray(fc_w, f32); fc_b = np.asarray(fc_b, f32)
    w_ih = np.asarray(w_ih, f32); w_hh = np.asarray(w_hh, f32)
    b_ih = np.asarray(b_ih, f32); b_hh = np.asarray(b_hh, f32)
    actor_w = np.asarray(actor_w, f32); actor_b = np.asarray(actor_b, f32)
    critic_w = np.asarray(critic_w, f32)
    critic_b = np.asarray(critic_b, f32)

    shared = {}
    # conv1 stationaries: S1[(g,rx)][p=(ry,c,dy,dx), (g',o)]
    w1s = w1 / 255.0
    S1 = np.zeros((4, 2, 128, 128), f32)
    for g in range(4):
        for rx in range(2):
            for ry in range(2):
                for c in range(C):
                    for dy in range(4):
                        for dx in range(4):
                            p = ry * 64 + c * 16 + dy * 4 + dx
                            S1[g, rx, p, g * 32:(g + 1) * 32] = \
                                w1s[:, c, 4 * ry + dy, 4 * rx + dx]
    shared["s1"] = S1.reshape(8, 128, 128)

    W2 = np.zeros((2, 2, 128, 128), f32)
    for ry in range(2):
        for rx in range(2):
            for g in range(4):
                dy2, dx2 = g // 2, g % 2
                for c in range(32):
                    col = w2[:, c, 2 * ry + dy2, 2 * rx + dx2]
                    W2[ry, rx, g * 32 + c, 0:64] = col
                    W2[ry, rx, g * 32 + c, 64:128] = col
    shared["w2"] = W2.reshape(4, 128, 128)

    W3A = np.zeros((3, 128, 128), f32)
    W3B = np.zeros((3, 64, 128), f32)
    for kx in range(3):
        for kyg in range(2):
            for c in range(64):
                for g4 in range(4):
                    W3A[kx, kyg * 64 + c, g4 * 32:(g4 + 1) * 32] = \
                        w3[:, c, kyg, kx]
        for c in range(64):
            for g4 in range(4):
                W3B[kx, c, g4 * 32:(g4 + 1) * 32] = w3[:, c, 2, kx]
    shared["w3a"] = W3A
    shared["w3b"] = W3B

    FCW = np.zeros((36, 4, 128, 128), f32)
    for j in range(36):
        for g in range(4):
            for c in range(32):
                FCW[j, :, g * 32 + c, :] = \
                    fc_w[:, c * 144 + 4 * j + g].reshape(4, 128)
    shared["fcw"] = FCW.reshape(144, 128, 128)

    GIW = np.zeros((4, 12, 128, 128), f32)
    WHH = np.zeros((4, 12, 128, 128), f32)
    for hc in range(4):
        for gc in range(12):
            GIW[hc, gc] = w_ih[gc * 128:(gc + 1) * 128,
                               hc * 128:(hc + 1) * 128].T
            WHH[hc, gc] = w_hh[gc * 128:(gc + 1) * 128,
                               hc * 128:(hc + 1) * 128].T
    shared["giw"] = GIW.reshape(48, 128, 128)
    shared["whh"] = WHH.reshape(48, 128, 128)

    ACm = np.concatenate([actor_w, critic_w], axis=0)       # [7, 512]
    ACW = np.zeros((4, 128, 7), f32)
    for hc in range(4):
        ACW[hc] = ACm[:, hc * 128:(hc + 1) * 128].T
    shared["acw"] = ACW

    shared["b1"] = np.tile(b1, 4).reshape(128, 1)
    shared["b2"] = np.tile(b2, 2).reshape(128, 1)
    shared["b3"] = np.tile(b3, 4).reshape(128, 1)
    shared["fcb"] = fc_b.reshape(4, 128).T.copy()
    gib = b_ih + np.concatenate([b_hh[:1024], np.zeros(512, f32)])
    shared["gib"] = gib.reshape(12, 128).T.copy()
    bhhn = b_hh[1024:].reshape(4, 128).T                     # [128, gc']
    shared["bhhn"] = np.repeat(bhhn, EPC, axis=1).copy()     # [(gc',e)]
    acb = np.concatenate([actor_b, critic_b]).astype(f32)
    shared["acb"] = np.tile(acb, (IPC, 1))

    for k in ("s1", "w2", "w3a", "w3b", "fcw", "giw", "whh"):
        shared[k] = shared[k].astype(BF16)

    # per-core tensors
    x_all = inputs.reshape(T, N, C, HW, HW)
    m_all = masks.reshape(T, N)
    a_all = np.asarray(action).reshape(T, N)
    in_maps = []
    # vectorized phase-plane build for all images at once:
    # P[(t,n), p=(ry,c,dy,dx), Y, X] = x_all[t, n, c, 4Y+4ry+dy, 4X+dx]
    xb = x_all.reshape(T * N, C, 31, 4, 31, 4)     # [i, c, Yr, dy, Xr, dx]
    P_all = np.zeros((T * N, 2, C, 4, 4, 30, 31), np.float32)
    for ry in range(2):
        P_all[:, ry] = xb[:, :, ry:ry + 30].transpose(
            0, 1, 3, 5, 2, 4)                       # [i, c, dy, dx, Y, X]
    P_all = P_all.reshape(T * N, 128, 930).astype(BF16)

    for k in range(NCORES):
        envs = [EPC * k + e for e in range(EPC)]
        idx = [t * N + n for t in range(T) for n in envs]
        m = dict(shared)
        m["p2"] = np.ascontiguousarray(P_all[idx])
        h0 = np.zeros((128, 4, EPC), np.float32)
        for e in range(EPC):
            h0[:, :, e] = states[envs[e]].reshape(4, 128).T
        m["h0"] = h0.reshape(128, 4 * EPC)
        mr = np.zeros((T, 4, EPC), np.float32)
        for e in range(EPC):
            mr[:, :, e] = m_all[:, envs[e]][:, None]
        m["mrep"] = np.broadcast_to(
            mr.reshape(1, T * 4 * EPC), (128, T * 4 * EPC)).copy()
        amk = np.zeros((IPC, A), np.float32)
        for t in range(T):
            for e in range(EPC):
                amk[t * EPC + e, int(a_all[t, envs[e]])] = 1.0
        m["am"] = amk
        in_maps.append(m)
    return in_maps


def assemble(results):
    value = np.zeros((T * N, 1), np.float32)
    alp = np.zeros((T * N,), np.float32)
    ent = np.zeros((T * N,), np.float32)
    st = np.zeros((N, H), np.float32)
    for k in range(NCORES):
        r = results[k]
        v = r["v"].reshape(T, EPC)
        a = r["alp"].reshape(T, EPC)
        e = r["ent"].reshape(T, EPC)
        for t in range(T):
            for ee in range(EPC):
                row = t * N + EPC * k + ee
                value[row, 0] = v[t, ee]
                alp[row] = a[t, ee]
                ent[row] = e[t, ee]
        stk = r["st"].reshape(128, 4, EPC)       # [p, hc, e]
        for e in range(EPC):
            st[EPC * k + e] = stk[:, :, e].T.reshape(H)
    return value, alp, ent, st


def kernel(**inputs):
    mo = bool(np.all(np.asarray(inputs["masks"]) == 1.0))
    key = ("nc", mo)
    if key not in _cache:
        _cache[key] = build_program(reps=1, masks_ones=mo)
    nc = _cache[key]
    in_maps = host_prep(**inputs)
    res = run_bass_kernel_spmd(nc, in_maps, list(range(NCORES)))
    return assemble(res.results)


# revision 17
# speedup vs baseline: 3.0888x; 1.1711x over previous
"""Trainium2 Bass kernel for CNN+GRU actor-critic (T=32, N=16 envs, H=512).

Sharding: data-parallel over envs — each of the 8 cores processes 2 envs
x 32 timesteps = 64 images through the CNN trunk + fc + input-gate GEMM,
then runs the GRU recurrence locally for its 2 envs, then the actor/critic
heads. All parameters are replicated. No collectives; the host scatters
inputs and gathers outputs.

Layout strategy (per core):
  * conv1 (8x8 s4) consumes host-prepared "phase planes"
      P2[p=(ry,c,dy,dx), Y, X] = img[c, 4(Y+ry)+dy, 4X+dx]
    so the 256-term contraction becomes 2 accumulating K=128 matmuls (rx).
    The stationary is padded to M=128 output columns (g',o) so conv1's
    output lands directly in conv2's phase layout [(dy2,dx2,o), Y2, X2].
  * conv2 (4x4 s2): 4 accumulating K=128 matmuls over (ry,rx); stationary
    columns duplicated (kyg,o) so eviction produces the row-shifted pair
    QQ[(kyg,c), Yq, X] = relu(conv2)[c, Yq+kyg, X] that conv3 needs.
  * conv3 (3x3 s1): ky in {0,1} packed on partitions (K=128), ky=2 as a
    K=64 matmul; stationary columns duplicated (g4,o) so eviction writes
    DUP[(g4,c), yxq, img] = relu(conv3)[c, 4*yxq+g4], which is exactly the
    fc moving operand for k-chunks of 128 = (4 spatial positions x 32 ch).
  * fc / gi: plain chunked GEMMs, images batched in the moving free dim.
  * GRU scan, "form S": stationary = w_hh.T chunks (48 LDW/step, bf16 FWL),
    moving = h.T [128,2]; gates land on partitions -> cheap pointwise.
  * heads: lhsT = GRU outputs [128, 64 imgs], moving = [actor;critic].T
    [128,7] fp32; log-softmax / entropy / gather pointwise on [64,7].

Matmul inputs are bf16 (PSUM accumulation fp32); scan pointwise, GI and
heads are fp32. Validated vs the jax reference at rel err ~1e-3 (value),
~5e-3 (states_out, which has ~1e-3 scale).
"""
import numpy as np
import ml_dtypes

import concourse.bass as bass
import concourse.tile as tile
from concourse import bacc, mybir
from concourse.bass_utils import run_bass_kernel_spmd

T, N, C, HW, A, H = 32, 16, 4, 124, 6, 512
NCORES = 8
EPC = N // NCORES            # 2 envs per core
IPC = T * EPC                # 64 images per core
BF16 = ml_dtypes.bfloat16

F32 = mybir.dt.float32
BF = mybir.dt.bfloat16
AF = mybir.ActivationFunctionType
OP = mybir.AluOpType
AX = mybir.AxisListType

_cache = {}


# ----------------------------------------------------------------------------
# device program
# ----------------------------------------------------------------------------

def build_program(reps: int = 1, n_imgs: int = IPC, do_fc: bool = True, do_scan: bool = True, do_heads: bool = True, masks_ones: bool = False):
    nc = bacc.Bacc("TRN2", target_bir_lowering=False, debug=False)

    di = {}

    def inp(name, shape, dt):
        di[name] = nc.dram_tensor(name, list(shape), dt, kind="ExternalInput")
        return di[name]

    def outp(name, shape, dt):
        di[name] = nc.dram_tensor(name, list(shape), dt, kind="ExternalOutput")
        return di[name]

    d_p2 = inp("p2", [IPC, 128, 930], BF)          # (30*31 free)
    d_s1 = inp("s1", [8, 128, 128], BF)            # (g*2+rx)
    d_w2 = inp("w2", [4, 128, 128], BF)            # (ry*2+rx)
    d_w3a = inp("w3a", [3, 128, 128], BF)          # kx
    d_w3b = inp("w3b", [3, 64, 128], BF)           # kx
    d_fcw = inp("fcw", [144, 128, 128], BF)        # (j*4+mc)
    d_giw = inp("giw", [48, 128, 128], BF)         # (hc*12+gc)
    d_whh = inp("whh", [48, 128, 128], BF)         # (hc*12+gc)
    d_acw = inp("acw", [4, 128, 7], F32)           # hc
    d_b1 = inp("b1", [128, 1], F32)
    d_b2 = inp("b2", [128, 1], F32)
    d_b3 = inp("b3", [128, 1], F32)
    d_fcb = inp("fcb", [128, 4], F32)
    d_gib = inp("gib", [128, 12], F32)
    d_bhhn = inp("bhhn", [128, 4 * EPC], F32)      # [(gc',e)] rep, gc'=0..3
    d_mrep = inp("mrep", [128, T * 4 * EPC], F32)  # [(t,hc,e)] rep over p,hc
    d_h0 = inp("h0", [128, 4 * EPC], F32)          # [(hc,e)]
    d_am = inp("am", [IPC, A], F32)                # one-hot(action)
    d_acb = inp("acb", [IPC, 7], F32)              # [actor_b, critic_b] rep

    d_v = outp("v", [IPC, 1], F32)
    d_alp = outp("alp", [IPC, 1], F32)
    d_ent = outp("ent", [IPC, 1], F32)
    d_st = outp("st", [128, 4 * EPC], F32)

    with tile.TileContext(nc) as tc:
        from contextlib import ExitStack
        with ExitStack() as ctx:
            cpool = ctx.enter_context(tc.tile_pool(name="consts", bufs=1))
            work = ctx.enter_context(tc.tile_pool(name="work", bufs=4))
            small = ctx.enter_context(tc.tile_pool(name="small", bufs=2))
            persist = ctx.enter_context(tc.tile_pool(name="persist", bufs=1))
            pscv = ctx.enter_context(
                tc.tile_pool(name="pscv", bufs=2, space="PSUM"))
            pssm = ctx.enter_context(
                tc.tile_pool(name="pssm", bufs=2, space="PSUM"))
            pshd = pssm

            # ---- load constants ----
            def cload(dram, shape, dt):
                t = cpool.tile(list(shape), dt, tag=dram.name)
                if len(dram.shape) == 3:
                    a, p, m = dram.shape
                    nc.sync.dma_start(
                        t[:].rearrange("p (a m) -> p a m", a=a),
                        dram.ap().rearrange("a p m -> p a m"))
                else:
                    nc.sync.dma_start(t[:], dram[:])
                return t

            s1 = cload(d_s1, [128, 8 * 128], BF)
            w2 = cload(d_w2, [128, 4 * 128], BF)
            w3a = cload(d_w3a, [128, 3 * 128], BF)
            w3b = cload(d_w3b, [64, 3 * 128], BF)
            fcw = cload(d_fcw, [128, 144 * 128], BF)
            giw = cload(d_giw, [128, 48 * 128], BF)
            whh = cload(d_whh, [128, 48 * 128], BF)
            acw = cload(d_acw, [128, 4 * 7], F32)
            b1 = cload(d_b1, [128, 1], F32)
            b2 = cload(d_b2, [128, 1], F32)
            b3 = cload(d_b3, [128, 1], F32)
            fcb = cload(d_fcb, [128, 4], F32)
            gib = cload(d_gib, [128, 12], F32)
            bhhn = cload(d_bhhn, [128, 4 * EPC], F32)
            mrep = cload(d_mrep, [128, T * 4 * EPC], F32)
            h0 = cload(d_h0, [128, 4 * EPC], F32)
            am = cload(d_am, [IPC, A], F32)
            acb = cload(d_acb, [IPC, 7], F32)

            def body(_iv=None):
                E = EPC
                DUP = persist.tile([128, 36 * IPC], BF, tag="dup")
                FC = persist.tile([128, 4 * IPC], BF, tag="fc")
                GI = persist.tile([128, 12 * IPC], F32, tag="gi")
                OUTS = persist.tile([128, 4 * T * E], F32, tag="outs")
                OUTSv = OUTS[:].rearrange("p (hc t e) -> p hc t e", hc=4, t=T)

                # ---- CNN trunk: 2 images per matmul, 1-pair stage skew ----
                P = 2
                NP = n_imgs // P
                dup_i = DUP[:].rearrange("p (j i) -> p i j", j=36)
                qs, ps3s, qqs, ps4s = {}, {}, {}, {}

                def st_conv1(ip):
                    i0 = ip * P
                    p2 = work.tile([128, P * 930], BF, tag="p2")
                    nc.sync.dma_start(
                        p2[:].rearrange("p (i f) -> p i f", i=P),
                        d_p2[i0:i0 + P].rearrange("i p f -> p i f"))
                    p2v = p2[:].rearrange("p (i y x) -> p i y x", i=P, y=30)
                    ps2 = pscv.tile([128, P * 225], F32, tag="c1")
                    k = 0
                    for g in range(4):
                        dy2, dx2 = g // 2, g % 2
                        for rx in range(2):
                            rhs = p2v[:, :, dy2:30:2,
                                      dx2 + rx:31:2][:, :, :15, :15]
                            nc.tensor.matmul(
                                ps2[:], s1[:, bass.ts(g * 2 + rx, 128)], rhs,
                                start=(k == 0), stop=(k == 7))
                            k += 1
                    q = work.tile([128, P * 225], BF, tag="q")
                    nc.scalar.activation(q[:], ps2[:], AF.Relu, bias=b1[:])
                    qs[ip] = q

                def st_conv2(ip):
                    qv = qs.pop(ip)[:].rearrange(
                        "p (i y x) -> p i y x", i=P, y=15)
                    ps3 = pscv.tile([128, P * 196], F32, tag="c2")
                    k = 0
                    for ry in range(2):
                        for rx in range(2):
                            rhs = qv[:, :, ry:ry + 14, rx:rx + 14]
                            nc.tensor.matmul(
                                ps3[:], w2[:, bass.ts(ry * 2 + rx, 128)], rhs,
                                start=(k == 0), stop=(k == 3))
                            k += 1
                    ps3v = ps3[:].rearrange("p (i y x) -> p i y x", i=P, y=14)
                    qq = work.tile([128, P * 196], BF, tag="qq")
                    qqv = qq[:].rearrange("p (i y x) -> p i y x", i=P, y=14)
                    nc.vector.tensor_scalar(qqv[0:64], ps3v[0:64],
                                            b2[0:64], 0.0,
                                            op0=OP.add, op1=OP.max)
                    nc.vector.tensor_scalar(qqv[64:128, :, 0:13],
                                            ps3v[64:128, :, 1:14],
                                            b2[64:128], 0.0,
                                            op0=OP.add, op1=OP.max)
                    qqs[ip] = qq

                def st_conv3(ip):
                    i0 = ip * P
                    qqv = qqs.pop(ip)[:].rearrange(
                        "p (i y x) -> p i y x", i=P, y=14)
                    ps4 = pscv.tile([128, P * 144], F32, tag="c3")
                    for kx in range(3):
                        nc.tensor.matmul(
                            ps4[:], w3a[:, bass.ts(kx, 128)],
                            qqv[:, :, 0:12, kx:kx + 12],
                            start=(kx == 0), stop=False)
                    for kx in range(3):
                        nc.tensor.matmul(
                            ps4[:], w3b[:, bass.ts(kx, 128)],
                            qqv[0:64, :, 2:14, kx:kx + 12],
                            start=False, stop=(kx == 2))
                    ps4v = ps4[:].rearrange("p (i yx) -> p i yx", i=P)
                    for g4 in range(4):
                        sl = slice(g4 * 32, (g4 + 1) * 32)
                        nc.scalar.activation(
                            dup_i[sl, i0:i0 + P, :],
                            ps4v[sl, :, g4:144:4],
                            AF.Relu, bias=b3[sl])

                dupv = DUP[:].rearrange("p (j i) -> p j i", j=36)
                HI = IPC // 2        # images per half

                def st_fc(half):
                    sl = slice(half * HI, (half + 1) * HI)
                    for mc in range(4):
                        psf = pssm.tile([128, HI], F32, tag="sm")
                        for j in range(36):
                            nc.tensor.matmul(
                                psf[:], fcw[:, bass.ts(j * 4 + mc, 128)],
                                dupv[:, j, sl],
                                start=(j == 0), stop=(j == 35))
                        nc.scalar.activation(
                            FC[:, mc * IPC + half * HI:
                               mc * IPC + half * HI + HI],
                            psf[:], AF.Relu, bias=fcb[:, mc:mc + 1])

                def st_gi(half):
                    for gc in range(12):
                        psg = pssm.tile([128, HI], F32, tag="sm")
                        for hc in range(4):
                            nc.tensor.matmul(
                                psg[:], giw[:, bass.ts(hc * 12 + gc, 128)],
                                FC[:, hc * IPC + half * HI:
                                   hc * IPC + half * HI + HI],
                                start=(hc == 0), stop=(hc == 3))
                        nc.scalar.activation(
                            GI[:, gc * IPC + half * HI:
                               gc * IPC + half * HI + HI],
                            psg[:], AF.Identity, bias=gib[:, gc:gc + 1])

                GIv = GI[:].rearrange("p (gc t e) -> p gc t e", gc=12, t=T)
                mrv = mrep[:].rearrange("p (t he) -> p t he", t=T)
                scan_state = {"zh": None, "m1": None}

                def finish_outputs_stub():
                    nc.sync.dma_start(d_v[:], am[:, 0:1])
                    nc.sync.dma_start(d_alp[:], am[:, 0:1])
                    nc.sync.dma_start(d_ent[:], am[:, 0:1])
                    nc.sync.dma_start(d_st[:], mrep[:, 0:4 * EPC])

                def scan_step(t):
                    E = EPC
                    hprev = h0[:] if t == 0 else OUTSv[:, :, t - 1, :]
                    if not masks_ones:
                        hm = small.tile([128, 4 * E], F32, tag="hm")
                        nc.vector.tensor_tensor(hm[:], hprev, mrv[:, t, :],
                                                op=OP.mult)
                        hm_ap = hm[:]
                    else:
                        hm_ap = hprev
                    hmb = small.tile([128, 4 * E], BF, tag="hmb")
                    if masks_ones and scan_state["zh"] is not None:
                        nc.vector.tensor_tensor(hmb[:], scan_state["zh"],
                                                scan_state["m1"], op=OP.add)
                    else:
                        nc.vector.tensor_copy(hmb[:], hm_ap)
                    hmbv = hmb[:].rearrange("p (hc e) -> p hc e", hc=4)

                    pss = pssm.tile([128, 12 * E], F32, tag="sm")
                    nc.vector.tensor_copy(pss[:, 0:8 * E], GIv[:, 0:8, t, :])
                    nc.vector.tensor_copy(pss[:, 8 * E:12 * E], bhhn[:])
                    for gc in range(12):
                        for hc in range(4):
                            nc.tensor.matmul(
                                pss[:, bass.ts(gc, E)],
                                whh[:, bass.ts(hc * 12 + gc, 128)],
                                hmbv[:, hc, :],
                                start=False, stop=(hc == 3),
                                skip_group_check=True)

                    rz = small.tile([128, 8 * E], F32, tag="rz")
                    nc.scalar.activation(rz[:], pss[:, 0:8 * E], AF.Sigmoid)
                    # h' = z*h + (1-z)*n, restructured so the bf16 state for
                    # the next step's matmuls is 2 ops past tanh.
                    zh = small.tile([128, 4 * E], F32, tag="zh")
                    nc.vector.tensor_tensor(zh[:], rz[:, 4 * E:8 * E], hm_ap,
                                            op=OP.mult)
                    zz = small.tile([128, 4 * E], F32, tag="zz")
                    nc.vector.tensor_scalar(zz[:], rz[:, 4 * E:8 * E],
                                            -1.0, 1.0, op0=OP.mult, op1=OP.add)
                    t1 = small.tile([128, 4 * E], F32, tag="t1")
                    nc.vector.tensor_tensor(t1[:], rz[:, 0:4 * E],
                                            pss[:, 8 * E:12 * E],
                                            op=OP.mult)
                    t2 = small.tile([128, 4 * E], F32, tag="t2")
                    nc.vector.tensor_tensor(t2[:], t1[:], GIv[:, 8:12, t, :],
                                            op=OP.add)
                    nn = small.tile([128, 4 * E], F32, tag="nn")
                    nc.scalar.activation(nn[:], t2[:], AF.Tanh)
                    m1 = small.tile([128, 4 * E], F32, tag="m1")
                    nc.vector.tensor_tensor(m1[:], zz[:], nn[:], op=OP.mult)
                    nc.gpsimd.tensor_tensor(OUTSv[:, :, t, :], zh[:], m1[:],
                                            op=OP.add)
                    scan_state["zh"], scan_state["m1"] = zh[:], m1[:]

                # ---------- emission schedule ----------
                NPH = NP // 2
                for ip in range(NPH + 2):        # conv half 0 (skewed)
                    if ip < NPH:
                        st_conv1(ip)
                    if 1 <= ip <= NPH:
                        st_conv2(ip - 1)
                    if 2 <= ip:
                        st_conv3(ip - 2)
                if not do_fc:
                    finish_outputs_stub()
                    return
                st_fc(0)
                st_gi(0)
                if not do_scan:
                    finish_outputs_stub()
                    return
                for k in range(NPH + 2):         # conv half 1 x scan 0..15
                    ip = NPH + k
                    if ip < NP:
                        st_conv1(ip)
                    if 1 <= k and ip - 1 < NP:
                        st_conv2(ip - 1)
                    if 2 <= k and ip - 2 < NP:
                        st_conv3(ip - 2)
                    if k < T // 2:
                        scan_step(k)
                st_fc(1)
                st_gi(1)
                for t in range(T // 2, T):
                    scan_step(t)

                # ---------------- heads ----------------
                if not do_heads:
                    nc.sync.dma_start(d_v[:], am[:, 0:1])
                    nc.sync.dma_start(d_alp[:], am[:, 0:1])
                    nc.sync.dma_start(d_ent[:], am[:, 0:1])
                    nc.sync.dma_start(
                        d_st.ap().rearrange("p (hc e) -> p hc e", hc=4),
                        OUTSv[:, :, T - 1, :])
                    return
                psl = pshd.tile([IPC, 7], F32, tag="sm")
                for hc in range(4):
                    nc.tensor.matmul(
                        psl[:], OUTSv[:, hc, :, :], acw[:, bass.ts(hc, 7)],
                        start=(hc == 0), stop=(hc == 3))
                lb = small.tile([IPC, 7], F32, tag="lb")
                nc.vector.tensor_tensor(lb[:], psl[:], acb[:], op=OP.add)
                mx = small.tile([IPC, 1], F32, tag="mx")
                nc.vector.tensor_reduce(mx[:], lb[:, 0:6], axis=AX.X,
                                        op=OP.max)
                sh = small.tile([IPC, 6], F32, tag="sh")
                nc.vector.tensor_scalar(sh[:], lb[:, 0:6], mx[:], None,
                                        op0=OP.subtract)
                ee = small.tile([IPC, 6], F32, tag="ee")
                ss = small.tile([IPC, 1], F32, tag="ss")
                nc.scalar.activation(ee[:], sh[:], AF.Exp, accum_out=ss[:])
                ls = small.tile([IPC, 1], F32, tag="ls")
                nc.scalar.activation(ls[:], ss[:], AF.Ln)
                logp = small.tile([IPC, 6], F32, tag="logp")
                nc.vector.tensor_scalar(logp[:], sh[:], ls[:], None,
                                        op0=OP.subtract)
                ta = small.tile([IPC, 6], F32, tag="ta")
                nc.vector.tensor_tensor(ta[:], logp[:], am[:], op=OP.mult)
                alp_t = small.tile([IPC, 1], F32, tag="alp")
                nc.vector.tensor_reduce(alp_t[:], ta[:], axis=AX.X, op=OP.add)
                rs = small.tile([IPC, 1], F32, tag="rs")
                nc.vector.reciprocal(rs[:], ss[:])
                pp = small.tile([IPC, 6], F32, tag="pp")
                nc.vector.tensor_scalar(pp[:], ee[:], rs[:], None,
                                        op0=OP.mult)
                tb = small.tile([IPC, 6], F32, tag="tb")
                nc.vector.tensor_tensor(tb[:], pp[:], logp[:], op=OP.mult)
                tbs = small.tile([IPC, 1], F32, tag="tbs")
                nc.vector.tensor_reduce(tbs[:], tb[:], axis=AX.X, op=OP.add)
                ent_t = small.tile([IPC, 1], F32, tag="ent")
                nc.vector.tensor_scalar(ent_t[:], tbs[:], -1.0, None,
                                        op0=OP.mult)
                vv = small.tile([IPC, 1], F32, tag="vv")
                nc.vector.tensor_copy(vv[:], lb[:, 6:7])

                nc.sync.dma_start(d_v[:], vv[:])
                nc.sync.dma_start(d_alp[:], alp_t[:])
                nc.sync.dma_start(d_ent[:], ent_t[:])
                nc.sync.dma_start(
                    d_st.ap().rearrange("p (hc e) -> p hc e", hc=4),
                    OUTSv[:, :, T - 1, :])

            if reps == 1:
                body()
            else:
                with tc.For_i(0, reps, 1) as _i:
                    body(_i)

    nc.compile()
    return nc


# ----------------------------------------------------------------------------
# host-side data prep (pure permutations / casts — no FLOPs beyond /255 fold)
# ----------------------------------------------------------------------------

def host_prep(inputs, states, masks, action, conv1_w, conv1_b, conv2_w,
              conv2_b, conv3_w, conv3_b, fc_w, fc_b, w_ih, w_hh, b_ih, b_hh,
              actor_w, actor_b, critic_w, critic_b):
    f32 = np.float32
    inputs = np.asarray(inputs, f32)
    states = np.asarray(states, f32)
    masks = np.asarray(masks, f32)
    action = np.asarray(action)
    w1 = np.asarray(conv1_w, f32); b1 = np.asarray(conv1_b, f32)
    w2 = np.asarray(conv2_w, f32); b2 = np.asarray(conv2_b, f32)
    w3 = np.asarray(conv3_w, f32); b3 = np.asarray(conv3_b, f32)
    fc_w = np.asarray(fc_w, f32); fc_b = np.asarray(fc_b, f32)
    w_ih = np.asarray(w_ih, f32); w_hh = np.asarray(w_hh, f32)
    b_ih = np.asarray(b_ih, f32); b_hh = np.asarray(b_hh, f32)
    actor_w = np.asarray(actor_w, f32); actor_b = np.asarray(actor_b, f32)
    critic_w = np.asarray(critic_w, f32)
    critic_b = np.asarray(critic_b, f32)

    shared = {}
    # conv1 stationaries: S1[(g,rx)][p=(ry,c,dy,dx), (g',o)]
    w1s = w1 / 255.0
    S1 = np.zeros((4, 2, 128, 128), f32)
    for g in range(4):
        for rx in range(2):
            for ry in range(2):
                for c in range(C):
                    for dy in range(4):
                        for dx in range(4):
                            p = ry * 64 + c * 16 + dy * 4 + dx
                            S1[g, rx, p, g * 32:(g + 1) * 32] = \
                                w1s[:, c, 4 * ry + dy, 4 * rx + dx]
    shared["s1"] = S1.reshape(8, 128, 128)

    W2 = np.zeros((2, 2, 128, 128), f32)
    for ry in range(2):
        for rx in range(2):
            for g in range(4):
                dy2, dx2 = g // 2, g % 2
                for c in range(32):
                    col = w2[:, c, 2 * ry + dy2, 2 * rx + dx2]
                    W2[ry, rx, g * 32 + c, 0:64] = col
                    W2[ry, rx, g * 32 + c, 64:128] = col
    shared["w2"] = W2.reshape(4, 128, 128)

    W3A = np.zeros((3, 128, 128), f32)
    W3B = np.zeros((3, 64, 128), f32)
    for kx in range(3):
        for kyg in range(2):
            for c in range(64):
                for g4 in range(4):
                    W3A[kx, kyg * 64 + c, g4 * 32:(g4 + 1) * 32] = \
                        w3[:, c, kyg, kx]
        for c in range(64):
            for g4 in range(4):
                W3B[kx, c, g4 * 32:(g4 + 1) * 32] = w3[:, c, 2, kx]
    shared["w3a"] = W3A
    shared["w3b"] = W3B

    FCW = np.zeros((36, 4, 128, 128), f32)
    for j in range(36):
        for g in range(4):
            for c in range(32):
                FCW[j, :, g * 32 + c, :] = \
                    fc_w[:, c * 144 + 4 * j + g].reshape(4, 128)
    shared["fcw"] = FCW.reshape(144, 128, 128)

    GIW = np.zeros((4, 12, 128, 128), f32)
    WHH = np.zeros((4, 12, 128, 128), f32)
    for hc in range(4):
        for gc in range(12):
            GIW[hc, gc] = w_ih[gc * 128:(gc + 1) * 128,
                               hc * 128:(hc + 1) * 128].T
            WHH[hc, gc] = w_hh[gc * 128:(gc + 1) * 128,
                               hc * 128:(hc + 1) * 128].T
    shared["giw"] = GIW.reshape(48, 128, 128)
    shared["whh"] = WHH.reshape(48, 128, 128)

    ACm = np.concatenate([actor_w, critic_w], axis=0)       # [7, 512]
    ACW = np.zeros((4, 128, 7), f32)
    for hc in range(4):
        ACW[hc] = ACm[:, hc * 128:(hc + 1) * 128].T
    shared["acw"] = ACW

    shared["b1"] = np.tile(b1, 4).reshape(128, 1)
    shared["b2"] = np.tile(b2, 2).reshape(128, 1)
    shared["b3"] = np.tile(b3, 4).reshape(128, 1)
    shared["fcb"] = fc_b.reshape(4, 128).T.copy()
    gib = b_ih + np.concatenate([b_hh[:1024], np.zeros(512, f32)])
    shared["gib"] = gib.reshape(12, 128).T.copy()
    bhhn = b_hh[1024:].reshape(4, 128).T                     # [128, gc']
    shared["bhhn"] = np.repeat(bhhn, EPC, axis=1).copy()     # [(gc',e)]
    acb = np.concatenate([actor_b, critic_b]).astype(f32)
    shared["acb"] = np.tile(acb, (IPC, 1))

    for k in ("s1", "w2", "w3a", "w3b", "fcw", "giw", "whh"):
        shared[k] = shared[k].astype(BF16)

    # per-core tensors
    x_all = inputs.reshape(T, N, C, HW, HW)
    m_all = masks.reshape(T, N)
    a_all = np.asarray(action).reshape(T, N)
    in_maps = []
    # vectorized phase-plane build for all images at once:
    # P[(t,n), p=(ry,c,dy,dx), Y, X] = x_all[t, n, c, 4Y+4ry+dy, 4X+dx]
    xb = x_all.reshape(T * N, C, 31, 4, 31, 4)     # [i, c, Yr, dy, Xr, dx]
    P_all = np.zeros((T * N, 2, C, 4, 4, 30, 31), np.float32)
    for ry in range(2):
        P_all[:, ry] = xb[:, :, ry:ry + 30].transpose(
            0, 1, 3, 5, 2, 4)                       # [i, c, dy, dx, Y, X]
    P_all = P_all.reshape(T * N, 128, 930).astype(BF16)

    for k in range(NCORES):
        envs = [EPC * k + e for e in range(EPC)]
        idx = [t * N + n for t in range(T) for n in envs]
        m = dict(shared)
        m["p2"] = np.ascontiguousarray(P_all[idx])
        h0 = np.zeros((128, 4, EPC), np.float32)
        for e in range(EPC):
            h0[:, :, e] = states[envs[e]].reshape(4, 128).T
        m["h0"] = h0.reshape(128, 4 * EPC)
        mr = np.zeros((T, 4, EPC), np.float32)
        for e in range(EPC):
            mr[:, :, e] = m_all[:, envs[e]][:, None]
        m["mrep"] = np.broadcast_to(
            mr.reshape(1, T * 4 * EPC), (128, T * 4 * EPC)).copy()
        amk = np.zeros((IPC, A), np.float32)
        for t in range(T):
            for e in range(EPC):
                amk[t * EPC + e, int(a_all[t, envs[e]])] = 1.0
        m["am"] = amk
        in_maps.append(m)
    return in_maps


def assemble(results):
    value = np.zeros((T * N, 1), np.float32)
    alp = np.zeros((T * N,), np.float32)
    ent = np.zeros((T * N,), np.float32)
    st = np.zeros((N, H), np.float32)
    for k in range(NCORES):
        r = results[k]
        v = r["v"].reshape(T, EPC)
        a = r["alp"].reshape(T, EPC)
        e = r["ent"].reshape(T, EPC)
        for t in range(T):
            for ee in range(EPC):
                row = t * N + EPC * k + ee
                value[row, 0] = v[t, ee]
                alp[row] = a[t, ee]
                ent[row] = e[t, ee]
        stk = r["st"].reshape(128, 4, EPC)       # [p, hc, e]
        for e in range(EPC):
            st[EPC * k + e] = stk[:, :, e].T.reshape(H)
    return value, alp, ent, st


def kernel(**inputs):
    mo = bool(np.all(np.asarray(inputs["masks"]) == 1.0))
    key = ("nc", mo)
    if key not in _cache:
        _cache[key] = build_program(reps=1, masks_ones=mo)
    nc = _cache[key]
    in_maps = host_prep(**inputs)
    res = run_bass_kernel_spmd(nc, in_maps, list(range(NCORES)))
    return assemble(res.results)


# revision 18
# speedup vs baseline: 3.1593x; 1.0228x over previous
"""Trainium2 Bass kernel for CNN+GRU actor-critic (T=32, N=16 envs, H=512).

Sharding: data-parallel over envs — each of the 8 cores processes 2 envs
x 32 timesteps = 64 images through the CNN trunk + fc + input-gate GEMM,
then runs the GRU recurrence locally for its 2 envs, then the actor/critic
heads. All parameters are replicated. No collectives; the host scatters
inputs and gathers outputs.

Layout strategy (per core):
  * conv1 (8x8 s4) consumes host-prepared "phase planes"
      P2[p=(ry,c,dy,dx), Y, X] = img[c, 4(Y+ry)+dy, 4X+dx]
    so the 256-term contraction becomes 2 accumulating K=128 matmuls (rx).
    The stationary is padded to M=128 output columns (g',o) so conv1's
    output lands directly in conv2's phase layout [(dy2,dx2,o), Y2, X2].
  * conv2 (4x4 s2): 4 accumulating K=128 matmuls over (ry,rx); stationary
    columns duplicated (kyg,o) so eviction produces the row-shifted pair
    QQ[(kyg,c), Yq, X] = relu(conv2)[c, Yq+kyg, X] that conv3 needs.
  * conv3 (3x3 s1): ky in {0,1} packed on partitions (K=128), ky=2 as a
    K=64 matmul; stationary columns duplicated (g4,o) so eviction writes
    DUP[(g4,c), yxq, img] = relu(conv3)[c, 4*yxq+g4], which is exactly the
    fc moving operand for k-chunks of 128 = (4 spatial positions x 32 ch).
  * fc / gi: plain chunked GEMMs, images batched in the moving free dim.
  * GRU scan, "form S": stationary = w_hh.T chunks (48 LDW/step, bf16 FWL),
    moving = h.T [128,2]; gates land on partitions -> cheap pointwise.
  * heads: lhsT = GRU outputs [128, 64 imgs], moving = [actor;critic].T
    [128,7] fp32; log-softmax / entropy / gather pointwise on [64,7].

Matmul inputs are bf16 (PSUM accumulation fp32); scan pointwise, GI and
heads are fp32. Validated vs the jax reference at rel err ~1e-3 (value),
~5e-3 (states_out, which has ~1e-3 scale).
"""
import numpy as np
import ml_dtypes

import concourse.bass as bass
import concourse.tile as tile
from concourse import bacc, mybir
from concourse.bass_utils import run_bass_kernel_spmd

T, N, C, HW, A, H = 32, 16, 4, 124, 6, 512
NCORES = 8
EPC = N // NCORES            # 2 envs per core
IPC = T * EPC                # 64 images per core
BF16 = ml_dtypes.bfloat16

F32 = mybir.dt.float32
BF = mybir.dt.bfloat16
AF = mybir.ActivationFunctionType
OP = mybir.AluOpType
AX = mybir.AxisListType

_cache = {}


# ----------------------------------------------------------------------------
# device program
# ----------------------------------------------------------------------------

def build_program(reps: int = 1, n_imgs: int = IPC, do_fc: bool = True, do_scan: bool = True, do_heads: bool = True, masks_ones: bool = False):
    nc = bacc.Bacc("TRN2", target_bir_lowering=False, debug=False)

    di = {}

    def inp(name, shape, dt):
        di[name] = nc.dram_tensor(name, list(shape), dt, kind="ExternalInput")
        return di[name]

    def outp(name, shape, dt):
        di[name] = nc.dram_tensor(name, list(shape), dt, kind="ExternalOutput")
        return di[name]

    d_p2 = inp("p2", [IPC, 128, 930], BF)          # (30*31 free)
    d_s1 = inp("s1", [8, 128, 128], BF)            # (g*2+rx)
    d_w2 = inp("w2", [4, 128, 128], BF)            # (ry*2+rx)
    d_w3a = inp("w3a", [3, 128, 128], BF)          # kx
    d_w3b = inp("w3b", [3, 64, 128], BF)           # kx
    d_fcw = inp("fcw", [144, 128, 128], BF)        # (j*4+mc)
    d_giw = inp("giw", [48, 128, 128], BF)         # (hc*12+gc)
    d_whh = inp("whh", [48, 128, 128], BF)         # (hc*12+gc)
    d_acw = inp("acw", [4, 128, 7], F32)           # hc
    d_b1 = inp("b1", [128, 1], F32)
    d_b2 = inp("b2", [128, 1], F32)
    d_b3 = inp("b3", [128, 1], F32)
    d_fcb = inp("fcb", [128, 4], F32)
    d_gib = inp("gib", [128, 12], F32)
    d_bhhn = inp("bhhn", [128, 4 * EPC], F32)      # [(gc',e)] rep, gc'=0..3
    d_mrep = inp("mrep", [128, T * 4 * EPC], F32)  # [(t,hc,e)] rep over p,hc
    d_h0 = inp("h0", [128, 4 * EPC], F32)          # [(hc,e)]
    d_am = inp("am", [IPC, A], F32)                # one-hot(action)
    d_acb = inp("acb", [IPC, 7], F32)              # [actor_b, critic_b] rep

    d_v = outp("v", [IPC, 1], F32)
    d_alp = outp("alp", [IPC, 1], F32)
    d_ent = outp("ent", [IPC, 1], F32)
    d_st = outp("st", [128, 4 * EPC], F32)

    with tile.TileContext(nc) as tc:
        from contextlib import ExitStack
        with ExitStack() as ctx:
            cpool = ctx.enter_context(tc.tile_pool(name="consts", bufs=1))
            work = ctx.enter_context(tc.tile_pool(name="work", bufs=6))
            small = ctx.enter_context(tc.tile_pool(name="small", bufs=3))
            persist = ctx.enter_context(tc.tile_pool(name="persist", bufs=1))
            pscv = ctx.enter_context(
                tc.tile_pool(name="pscv", bufs=2, space="PSUM"))
            pssm = ctx.enter_context(
                tc.tile_pool(name="pssm", bufs=2, space="PSUM"))
            pshd = pssm

            # ---- load constants ----
            def cload(dram, shape, dt):
                t = cpool.tile(list(shape), dt, tag=dram.name)
                if len(dram.shape) == 3:
                    a, p, m = dram.shape
                    nc.sync.dma_start(
                        t[:].rearrange("p (a m) -> p a m", a=a),
                        dram.ap().rearrange("a p m -> p a m"))
                else:
                    nc.sync.dma_start(t[:], dram[:])
                return t

            s1 = cload(d_s1, [128, 8 * 128], BF)
            w2 = cload(d_w2, [128, 4 * 128], BF)
            w3a = cload(d_w3a, [128, 3 * 128], BF)
            w3b = cload(d_w3b, [64, 3 * 128], BF)
            fcw = cload(d_fcw, [128, 144 * 128], BF)
            giw = cload(d_giw, [128, 48 * 128], BF)
            whh = cload(d_whh, [128, 48 * 128], BF)
            acw = cload(d_acw, [128, 4 * 7], F32)
            b1 = cload(d_b1, [128, 1], F32)
            b2 = cload(d_b2, [128, 1], F32)
            b3 = cload(d_b3, [128, 1], F32)
            fcb = cload(d_fcb, [128, 4], F32)
            gib = cload(d_gib, [128, 12], F32)
            bhhn = cload(d_bhhn, [128, 4 * EPC], F32)
            mrep = cload(d_mrep, [128, T * 4 * EPC], F32)
            h0 = cload(d_h0, [128, 4 * EPC], F32)
            am = cload(d_am, [IPC, A], F32)
            acb = cload(d_acb, [IPC, 7], F32)

            def body(_iv=None):
                E = EPC
                DUP = persist.tile([128, 36 * IPC], BF, tag="dup")
                FC = persist.tile([128, 4 * IPC], BF, tag="fc")
                GI = persist.tile([128, 12 * IPC], F32, tag="gi")
                OUTS = persist.tile([128, 4 * T * E], F32, tag="outs")
                OUTSv = OUTS[:].rearrange("p (hc t e) -> p hc t e", hc=4, t=T)

                # ---- CNN trunk: 2 images per matmul, 1-pair stage skew ----
                P = 2
                NP = n_imgs // P
                dup_i = DUP[:].rearrange("p (j i) -> p i j", j=36)
                qs, ps3s, qqs, ps4s = {}, {}, {}, {}

                def st_conv1(ip):
                    i0 = ip * P
                    p2 = work.tile([128, P * 930], BF, tag="p2")
                    nc.sync.dma_start(
                        p2[:].rearrange("p (i f) -> p i f", i=P),
                        d_p2[i0:i0 + P].rearrange("i p f -> p i f"))
                    p2v = p2[:].rearrange("p (i y x) -> p i y x", i=P, y=30)
                    ps2 = pscv.tile([128, P * 225], F32, tag="c1")
                    k = 0
                    for g in range(4):
                        dy2, dx2 = g // 2, g % 2
                        for rx in range(2):
                            rhs = p2v[:, :, dy2:30:2,
                                      dx2 + rx:31:2][:, :, :15, :15]
                            nc.tensor.matmul(
                                ps2[:], s1[:, bass.ts(g * 2 + rx, 128)], rhs,
                                start=(k == 0), stop=(k == 7))
                            k += 1
                    q = work.tile([128, P * 225], BF, tag="q")
                    nc.scalar.activation(q[:], ps2[:], AF.Relu, bias=b1[:])
                    qs[ip] = q

                def st_conv2(ip):
                    qv = qs.pop(ip)[:].rearrange(
                        "p (i y x) -> p i y x", i=P, y=15)
                    ps3 = pscv.tile([128, P * 196], F32, tag="c2")
                    k = 0
                    for ry in range(2):
                        for rx in range(2):
                            rhs = qv[:, :, ry:ry + 14, rx:rx + 14]
                            nc.tensor.matmul(
                                ps3[:], w2[:, bass.ts(ry * 2 + rx, 128)], rhs,
                                start=(k == 0), stop=(k == 3))
                            k += 1
                    ps3v = ps3[:].rearrange("p (i y x) -> p i y x", i=P, y=14)
                    qq = work.tile([128, P * 196], BF, tag="qq")
                    qqv = qq[:].rearrange("p (i y x) -> p i y x", i=P, y=14)
                    nc.vector.tensor_scalar(qqv[0:64], ps3v[0:64],
                                            b2[0:64], 0.0,
                                            op0=OP.add, op1=OP.max)
                    nc.vector.tensor_scalar(qqv[64:128, :, 0:13],
                                            ps3v[64:128, :, 1:14],
                                            b2[64:128], 0.0,
                                            op0=OP.add, op1=OP.max)
                    qqs[ip] = qq

                def st_conv3(ip):
                    i0 = ip * P
                    qqv = qqs.pop(ip)[:].rearrange(
                        "p (i y x) -> p i y x", i=P, y=14)
                    ps4 = pscv.tile([128, P * 144], F32, tag="c3")
                    for kx in range(3):
                        nc.tensor.matmul(
                            ps4[:], w3a[:, bass.ts(kx, 128)],
                            qqv[:, :, 0:12, kx:kx + 12],
                            start=(kx == 0), stop=False)
                    for kx in range(3):
                        nc.tensor.matmul(
                            ps4[:], w3b[:, bass.ts(kx, 128)],
                            qqv[0:64, :, 2:14, kx:kx + 12],
                            start=False, stop=(kx == 2))
                    ps4v = ps4[:].rearrange("p (i yx) -> p i yx", i=P)
                    for g4 in range(4):
                        sl = slice(g4 * 32, (g4 + 1) * 32)
                        nc.scalar.activation(
                            dup_i[sl, i0:i0 + P, :],
                            ps4v[sl, :, g4:144:4],
                            AF.Relu, bias=b3[sl])

                dupv = DUP[:].rearrange("p (j i) -> p j i", j=36)
                HI = IPC // 2        # images per half

                def st_fc(half):
                    sl = slice(half * HI, (half + 1) * HI)
                    for mc in range(4):
                        psf = pssm.tile([128, HI], F32, tag="sm")
                        for j in range(36):
                            nc.tensor.matmul(
                                psf[:], fcw[:, bass.ts(j * 4 + mc, 128)],
                                dupv[:, j, sl],
                                start=(j == 0), stop=(j == 35))
                        nc.scalar.activation(
                            FC[:, mc * IPC + half * HI:
                               mc * IPC + half * HI + HI],
                            psf[:], AF.Relu, bias=fcb[:, mc:mc + 1])

                def st_gi(half):
                    for gc in range(12):
                        psg = pssm.tile([128, HI], F32, tag="sm")
                        for hc in range(4):
                            nc.tensor.matmul(
                                psg[:], giw[:, bass.ts(hc * 12 + gc, 128)],
                                FC[:, hc * IPC + half * HI:
                                   hc * IPC + half * HI + HI],
                                start=(hc == 0), stop=(hc == 3))
                        nc.scalar.activation(
                            GI[:, gc * IPC + half * HI:
                               gc * IPC + half * HI + HI],
                            psg[:], AF.Identity, bias=gib[:, gc:gc + 1])

                GIv = GI[:].rearrange("p (gc t e) -> p gc t e", gc=12, t=T)
                mrv = mrep[:].rearrange("p (t he) -> p t he", t=T)
                scan_state = {"zh": None, "m1": None}

                def finish_outputs_stub():
                    nc.sync.dma_start(d_v[:], am[:, 0:1])
                    nc.sync.dma_start(d_alp[:], am[:, 0:1])
                    nc.sync.dma_start(d_ent[:], am[:, 0:1])
                    nc.sync.dma_start(d_st[:], mrep[:, 0:4 * EPC])

                def scan_step(t):
                    E = EPC
                    hprev = h0[:] if t == 0 else OUTSv[:, :, t - 1, :]
                    if not masks_ones:
                        hm = small.tile([128, 4 * E], F32, tag="hm")
                        nc.vector.tensor_tensor(hm[:], hprev, mrv[:, t, :],
                                                op=OP.mult)
                        hm_ap = hm[:]
                    else:
                        hm_ap = hprev
                    hmb = small.tile([128, 4 * E], BF, tag="hmb")
                    if masks_ones and scan_state["zh"] is not None:
                        nc.vector.tensor_tensor(hmb[:], scan_state["zh"],
                                                scan_state["m1"], op=OP.add)
                    else:
                        nc.vector.tensor_copy(hmb[:], hm_ap)
                    hmbv = hmb[:].rearrange("p (hc e) -> p hc e", hc=4)

                    pss = pssm.tile([128, 12 * E], F32, tag="sm")
                    nc.vector.tensor_copy(pss[:, 0:8 * E], GIv[:, 0:8, t, :])
                    nc.vector.tensor_copy(pss[:, 8 * E:12 * E], bhhn[:])
                    for gc in range(12):
                        for hc in range(4):
                            nc.tensor.matmul(
                                pss[:, bass.ts(gc, E)],
                                whh[:, bass.ts(hc * 12 + gc, 128)],
                                hmbv[:, hc, :],
                                start=False, stop=(hc == 3),
                                skip_group_check=True)

                    rz = small.tile([128, 8 * E], F32, tag="rz")
                    nc.scalar.activation(rz[:], pss[:, 0:8 * E], AF.Sigmoid)
                    # h' = z*h + (1-z)*n, restructured so the bf16 state for
                    # the next step's matmuls is 2 ops past tanh.
                    zh = small.tile([128, 4 * E], F32, tag="zh")
                    nc.vector.tensor_tensor(zh[:], rz[:, 4 * E:8 * E], hm_ap,
                                            op=OP.mult)
                    zz = small.tile([128, 4 * E], F32, tag="zz")
                    nc.vector.tensor_scalar(zz[:], rz[:, 4 * E:8 * E],
                                            -1.0, 1.0, op0=OP.mult, op1=OP.add)
                    t1 = small.tile([128, 4 * E], F32, tag="t1")
                    nc.vector.tensor_tensor(t1[:], rz[:, 0:4 * E],
                                            pss[:, 8 * E:12 * E],
                                            op=OP.mult)
                    t2 = small.tile([128, 4 * E], F32, tag="t2")
                    nc.vector.tensor_tensor(t2[:], t1[:], GIv[:, 8:12, t, :],
                                            op=OP.add)
                    nn = small.tile([128, 4 * E], F32, tag="nn")
                    nc.scalar.activation(nn[:], t2[:], AF.Tanh)
                    m1 = small.tile([128, 4 * E], F32, tag="m1")
                    nc.vector.tensor_tensor(m1[:], zz[:], nn[:], op=OP.mult)
                    nc.gpsimd.tensor_tensor(OUTSv[:, :, t, :], zh[:], m1[:],
                                            op=OP.add)
                    scan_state["zh"], scan_state["m1"] = zh[:], m1[:]

                # ---------- emission schedule ----------
                NPH = NP // 2
                for ip in range(NPH + 2):        # conv half 0 (skewed)
                    if ip < NPH:
                        st_conv1(ip)
                    if 1 <= ip <= NPH:
                        st_conv2(ip - 1)
                    if 2 <= ip:
                        st_conv3(ip - 2)
                if not do_fc:
                    finish_outputs_stub()
                    return
                st_fc(0)
                st_gi(0)
                if not do_scan:
                    finish_outputs_stub()
                    return
                for k in range(NPH + 2):         # conv half 1 x scan 0..15
                    ip = NPH + k
                    if ip < NP:
                        st_conv1(ip)
                    if 1 <= k and ip - 1 < NP:
                        st_conv2(ip - 1)
                    if 2 <= k and ip - 2 < NP:
                        st_conv3(ip - 2)
                    if k < T // 2:
                        scan_step(k)
                st_fc(1)
                st_gi(1)
                for t in range(T // 2, T):
                    scan_step(t)

                # ---------------- heads ----------------
                if not do_heads:
                    nc.sync.dma_start(d_v[:], am[:, 0:1])
                    nc.sync.dma_start(d_alp[:], am[:, 0:1])
                    nc.sync.dma_start(d_ent[:], am[:, 0:1])
                    nc.sync.dma_start(
                        d_st.ap().rearrange("p (hc e) -> p hc e", hc=4),
                        OUTSv[:, :, T - 1, :])
                    return
                psl = pshd.tile([IPC, 7], F32, tag="sm")
                for hc in range(4):
                    nc.tensor.matmul(
                        psl[:], OUTSv[:, hc, :, :], acw[:, bass.ts(hc, 7)],
                        start=(hc == 0), stop=(hc == 3))
                lb = small.tile([IPC, 7], F32, tag="lb")
                nc.vector.tensor_tensor(lb[:], psl[:], acb[:], op=OP.add)
                mx = small.tile([IPC, 1], F32, tag="mx")
                nc.vector.tensor_reduce(mx[:], lb[:, 0:6], axis=AX.X,
                                        op=OP.max)
                sh = small.tile([IPC, 6], F32, tag="sh")
                nc.vector.tensor_scalar(sh[:], lb[:, 0:6], mx[:], None,
                                        op0=OP.subtract)
                ee = small.tile([IPC, 6], F32, tag="ee")
                ss = small.tile([IPC, 1], F32, tag="ss")
                nc.scalar.activation(ee[:], sh[:], AF.Exp, accum_out=ss[:])
                ls = small.tile([IPC, 1], F32, tag="ls")
                nc.scalar.activation(ls[:], ss[:], AF.Ln)
                logp = small.tile([IPC, 6], F32, tag="logp")
                nc.vector.tensor_scalar(logp[:], sh[:], ls[:], None,
                                        op0=OP.subtract)
                ta = small.tile([IPC, 6], F32, tag="ta")
                nc.vector.tensor_tensor(ta[:], logp[:], am[:], op=OP.mult)
                alp_t = small.tile([IPC, 1], F32, tag="alp")
                nc.vector.tensor_reduce(alp_t[:], ta[:], axis=AX.X, op=OP.add)
                rs = small.tile([IPC, 1], F32, tag="rs")
                nc.vector.reciprocal(rs[:], ss[:])
                pp = small.tile([IPC, 6], F32, tag="pp")
                nc.vector.tensor_scalar(pp[:], ee[:], rs[:], None,
                                        op0=OP.mult)
                tb = small.tile([IPC, 6], F32, tag="tb")
                nc.vector.tensor_tensor(tb[:], pp[:], logp[:], op=OP.mult)
                tbs = small.tile([IPC, 1], F32, tag="tbs")
                nc.vector.tensor_reduce(tbs[:], tb[:], axis=AX.X, op=OP.add)
                ent_t = small.tile([IPC, 1], F32, tag="ent")
                nc.vector.tensor_scalar(ent_t[:], tbs[:], -1.0, None,
                                        op0=OP.mult)
                vv = small.tile([IPC, 1], F32, tag="vv")
                nc.vector.tensor_copy(vv[:], lb[:, 6:7])

                nc.sync.dma_start(d_v[:], vv[:])
                nc.sync.dma_start(d_alp[:], alp_t[:])
                nc.sync.dma_start(d_ent[:], ent_t[:])
                nc.sync.dma_start(
                    d_st.ap().rearrange("p (hc e) -> p hc e", hc=4),
                    OUTSv[:, :, T - 1, :])

            if reps == 1:
                body()
            else:
                with tc.For_i(0, reps, 1) as _i:
                    body(_i)

    nc.compile()
    return nc


# ----------------------------------------------------------------------------
# host-side data prep (pure permutations / casts — no FLOPs beyond /255 fold)
# ----------------------------------------------------------------------------

def host_prep(inputs, states, masks, action, conv1_w, conv1_b, conv2_w,
              conv2_b, conv3_w, conv3_b, fc_w, fc_b, w_ih, w_hh, b_ih, b_hh,
              actor_w, actor_b, critic_w, critic_b):
    f32 = np.float32
    inputs = np.asarray(inputs, f32)
    states = np.asarray(states, f32)
    masks = np.asarray(masks, f32)
    action = np.asarray(action)
    w1 = np.asarray(conv1_w, f32); b1 = np.asarray(conv1_b, f32)
    w2 = np.asarray(conv2_w, f32); b2 = np.asarray(conv2_b, f32)
    w3 = np.asarray(conv3_w, f32); b3 = np.asarray(conv3_b, f32)
    fc_w = np.asarray(fc_w, f32); fc_b = np.asarray(fc_b, f32)
    w_ih = np.asarray(w_ih, f32); w_hh = np.asarray(w_hh, f32)
    b_ih = np.asarray(b_ih, f32); b_hh = np.asarray(b_hh, f32)
    actor_w = np.asarray(actor_w, f32); actor_b = np.asarray(actor_b, f32)
    critic_w = np.asarray(critic_w, f32)
    critic_b = np.asarray(critic_b, f32)

    shared = {}
    # conv1 stationaries: S1[(g,rx)][p=(ry,c,dy,dx), (g',o)]
    w1s = w1 / 255.0
    S1 = np.zeros((4, 2, 128, 128), f32)
    for g in range(4):
        for rx in range(2):
            for ry in range(2):
                for c in range(C):
                    for dy in range(4):
                        for dx in range(4):
                            p = ry * 64 + c * 16 + dy * 4 + dx
                            S1[g, rx, p, g * 32:(g + 1) * 32] = \
                                w1s[:, c, 4 * ry + dy, 4 * rx + dx]
    shared["s1"] = S1.reshape(8, 128, 128)

    W2 = np.zeros((2, 2, 128, 128), f32)
    for ry in range(2):
        for rx in range(2):
            for g in range(4):
                dy2, dx2 = g // 2, g % 2
                for c in range(32):
                    col = w2[:, c, 2 * ry + dy2, 2 * rx + dx2]
                    W2[ry, rx, g * 32 + c, 0:64] = col
                    W2[ry, rx, g * 32 + c, 64:128] = col
    shared["w2"] = W2.reshape(4, 128, 128)

    W3A = np.zeros((3, 128, 128), f32)
    W3B = np.zeros((3, 64, 128), f32)
    for kx in range(3):
        for kyg in range(2):
            for c in range(64):
                for g4 in range(4):
                    W3A[kx, kyg * 64 + c, g4 * 32:(g4 + 1) * 32] = \
                        w3[:, c, kyg, kx]
        for c in range(64):
            for g4 in range(4):
                W3B[kx, c, g4 * 32:(g4 + 1) * 32] = w3[:, c, 2, kx]
    shared["w3a"] = W3A
    shared["w3b"] = W3B

    FCW = np.zeros((36, 4, 128, 128), f32)
    for j in range(36):
        for g in range(4):
            for c in range(32):
                FCW[j, :, g * 32 + c, :] = \
                    fc_w[:, c * 144 + 4 * j + g].reshape(4, 128)
    shared["fcw"] = FCW.reshape(144, 128, 128)

    GIW = np.zeros((4, 12, 128, 128), f32)
    WHH = np.zeros((4, 12, 128, 128), f32)
    for hc in range(4):
        for gc in range(12):
            GIW[hc, gc] = w_ih[gc * 128:(gc + 1) * 128,
                               hc * 128:(hc + 1) * 128].T
            WHH[hc, gc] = w_hh[gc * 128:(gc + 1) * 128,
                               hc * 128:(hc + 1) * 128].T
    shared["giw"] = GIW.reshape(48, 128, 128)
    shared["whh"] = WHH.reshape(48, 128, 128)

    ACm = np.concatenate([actor_w, critic_w], axis=0)       # [7, 512]
    ACW = np.zeros((4, 128, 7), f32)
    for hc in range(4):
        ACW[hc] = ACm[:, hc * 128:(hc + 1) * 128].T
    shared["acw"] = ACW

    shared["b1"] = np.tile(b1, 4).reshape(128, 1)
    shared["b2"] = np.tile(b2, 2).reshape(128, 1)
    shared["b3"] = np.tile(b3, 4).reshape(128, 1)
    shared["fcb"] = fc_b.reshape(4, 128).T.copy()
    gib = b_ih + np.concatenate([b_hh[:1024], np.zeros(512, f32)])
    shared["gib"] = gib.reshape(12, 128).T.copy()
    bhhn = b_hh[1024:].reshape(4, 128).T                     # [128, gc']
    shared["bhhn"] = np.repeat(bhhn, EPC, axis=1).copy()     # [(gc',e)]
    acb = np.concatenate([actor_b, critic_b]).astype(f32)
    shared["acb"] = np.tile(acb, (IPC, 1))

    for k in ("s1", "w2", "w3a", "w3b", "fcw", "giw", "whh"):
        shared[k] = shared[k].astype(BF16)

    # per-core tensors
    x_all = inputs.reshape(T, N, C, HW, HW)
    m_all = masks.reshape(T, N)
    a_all = np.asarray(action).reshape(T, N)
    in_maps = []
    # vectorized phase-plane build for all images at once:
    # P[(t,n), p=(ry,c,dy,dx), Y, X] = x_all[t, n, c, 4Y+4ry+dy, 4X+dx]
    xb = x_all.reshape(T * N, C, 31, 4, 31, 4)     # [i, c, Yr, dy, Xr, dx]
    P_all = np.zeros((T * N, 2, C, 4, 4, 30, 31), np.float32)
    for ry in range(2):
        P_all[:, ry] = xb[:, :, ry:ry + 30].transpose(
            0, 1, 3, 5, 2, 4)                       # [i, c, dy, dx, Y, X]
    P_all = P_all.reshape(T * N, 128, 930).astype(BF16)

    for k in range(NCORES):
        envs = [EPC * k + e for e in range(EPC)]
        idx = [t * N + n for t in range(T) for n in envs]
        m = dict(shared)
        m["p2"] = np.ascontiguousarray(P_all[idx])
        h0 = np.zeros((128, 4, EPC), np.float32)
        for e in range(EPC):
            h0[:, :, e] = states[envs[e]].reshape(4, 128).T
        m["h0"] = h0.reshape(128, 4 * EPC)
        mr = np.zeros((T, 4, EPC), np.float32)
        for e in range(EPC):
            mr[:, :, e] = m_all[:, envs[e]][:, None]
        m["mrep"] = np.broadcast_to(
            mr.reshape(1, T * 4 * EPC), (128, T * 4 * EPC)).copy()
        amk = np.zeros((IPC, A), np.float32)
        for t in range(T):
            for e in range(EPC):
                amk[t * EPC + e, int(a_all[t, envs[e]])] = 1.0
        m["am"] = amk
        in_maps.append(m)
    return in_maps


def assemble(results):
    value = np.zeros((T * N, 1), np.float32)
    alp = np.zeros((T * N,), np.float32)
    ent = np.zeros((T * N,), np.float32)
    st = np.zeros((N, H), np.float32)
    for k in range(NCORES):
        r = results[k]
        v = r["v"].reshape(T, EPC)
        a = r["alp"].reshape(T, EPC)
        e = r["ent"].reshape(T, EPC)
        for t in range(T):
            for ee in range(EPC):
                row = t * N + EPC * k + ee
                value[row, 0] = v[t, ee]
                alp[row] = a[t, ee]
                ent[row] = e[t, ee]
        stk = r["st"].reshape(128, 4, EPC)       # [p, hc, e]
        for e in range(EPC):
            st[EPC * k + e] = stk[:, :, e].T.reshape(H)
    return value, alp, ent, st


def kernel(**inputs):
    mo = bool(np.all(np.asarray(inputs["masks"]) == 1.0))
    key = ("nc", mo)
    if key not in _cache:
        _cache[key] = build_program(reps=1, masks_ones=mo)
    nc = _cache[key]
    in_maps = host_prep(**inputs)
    res = run_bass_kernel_spmd(nc, in_maps, list(range(NCORES)))
    return assemble(res.results)
